# revision 30
# baseline (speedup 1.0000x reference)
"""Trainium2 Bass kernel for BatchEnsemble encoder-decoder multihead attention.

Problem (hardcoded shapes): Tq=Tk=1024, B=8, H=1024, heads=16, hd=64.

Sharding: pure data parallelism - batch B=8 across the 8 NeuronCores, one
batch element per core. No collectives.

Per-core math (batch b), BatchEnsemble rank-1 factors and the 1/sqrt(hd)
scale folded into per-core bf16 weights on the host:
    Q^T = Wq''^T.T @ Xq^T          [H, Tq]  (head-pair dims on partitions)
    K^T = Wk''^T.T @ Xk^T          [H, Tk]
    V   = Xk^T.T @ Wv''^T          [Tk, H]  (natural layout, [V|1] stationary)
    per head g:
      S^T = K_g^T.T @ Q_g^T        [Tk, Tq]   bf16 matmuls
      E   = exp(S^T) -> bf16       (scores bounded, no max-subtraction)
      [ctx | den] = E_tile.T @ [V_g | 1]   [Tq-tile, 65]  <- transposed ctx:
            stationary = E tile (128 Tk x 128 Tq), moving = [V|1]: full PE
            utilization (the ones column gives the softmax denominator).
      ctx = ctx * (1/den)          (DVE per-partition scalar, no broadcast DMA)
    ctx^T via PE transpose (identity RHS, bf16)  -> [H, Tq]
    out  = ctx^T.T @ Wo^T          [Tq, H]

All matmul operands bf16 (1 cycle/row on the PE, fp32 PSUM accumulate);
measured end-to-end absmax error ~5e-3 of output scale.

Schedule: the attention inner loop is ACT(exp)-paced, so projection /
out-projection / transpose matmuls are emitted as fine-grained "fillers"
(a few matmuls per attention i-step) to keep the PE busy during exp waits.
ctx accumulators for 4 Tq-subtiles share one PSUM bank (regions at 65-col
offsets, lazy-zero semantics with skip_group_check on regions 1-3).
Out-projection for the first Tq half runs during the second half's
attention.
"""

from collections import deque

import numpy as np
import ml_dtypes

import concourse.bass as bass
import concourse.tile as tile
import concourse.mybir as mybir
from concourse import bacc
from concourse.bass_utils import run_bass_kernel_spmd

F32 = mybir.dt.float32
BF16 = mybir.dt.bfloat16
AF = mybir.ActivationFunctionType
NPBF = ml_dtypes.bfloat16

T = 1024        # Tq = Tk
H = 1024
B = 8
HEADS = 16
HD = 64
NT = T // 128   # 8 x 128-tiles
NB = T // 512   # 2 x 512-blocks (qb)
PAIRS = HEADS // 2

_cache = {}
_last_in_maps = None


def _build(with_bq, with_bk, with_bv):
    nc = bacc.Bacc("TRN2", target_bir_lowering=False, debug=False)

    xqt_d = nc.dram_tensor("xqt", [H, T], BF16, kind="ExternalInput")
    xkt_d = nc.dram_tensor("xkt", [H, T], BF16, kind="ExternalInput")
    wqt_d = nc.dram_tensor("wqt", [H, H], BF16, kind="ExternalInput")
    wkt_d = nc.dram_tensor("wkt", [H, H], BF16, kind="ExternalInput")
    wvt_d = nc.dram_tensor("wvt", [H, H], BF16, kind="ExternalInput")
    wot_d = nc.dram_tensor("wot", [H, H], BF16, kind="ExternalInput")
    id_d = nc.dram_tensor("ident", [128, 128], BF16, kind="ExternalInput")
    bq_d = nc.dram_tensor("bq", [H], F32, kind="ExternalInput") if with_bq else None
    bk_d = nc.dram_tensor("bk", [H], F32, kind="ExternalInput") if with_bk else None
    bv_d = nc.dram_tensor("bv", [H], F32, kind="ExternalInput") if with_bv else None
    out_d = nc.dram_tensor("out", [T, H], F32, kind="ExternalOutput")

    with tile.TileContext(nc) as tc:
        with tc.tile_pool(name="px", bufs=16) as px, \
             tc.tile_pool(name="pw", bufs=32) as pw, \
             tc.tile_pool(name="pq", bufs=8) as pq, \
             tc.tile_pool(name="pk", bufs=8) as pk, \
             tc.tile_pool(name="pv", bufs=8) as pv, \
             tc.tile_pool(name="pex", bufs=6) as pex, \
             tc.tile_pool(name="pcs", bufs=8) as pcs, \
             tc.tile_pool(name="pct", bufs=8) as pct, \
             tc.tile_pool(name="pou", bufs=3) as pou, \
             tc.tile_pool(name="prc", bufs=6) as prc, \
             tc.tile_pool(name="pms", bufs=4) as pms, \
             tc.tile_pool(name="dscr", bufs=2, space="DRAM") as dscr, \
             tc.tile_pool(name="psS", bufs=2, space="PSUM") as psS, \
             tc.tile_pool(name="psC", bufs=3, space="PSUM") as psC, \
             tc.tile_pool(name="psF", bufs=1, space="PSUM") as psF:

            # ---- input DMAs, consumption order ----
            def dma_in(tile_, src, h):
                # split the first h-tile into column halves so the first wave
                # matmul can start on half the bytes
                if h == 0:
                    nc.sync.dma_start(out=tile_[:, 0:512], in_=src[:, 0:512])
                    nc.sync.dma_start(out=tile_[:, 512:1024], in_=src[:, 512:1024])
                else:
                    nc.sync.dma_start(out=tile_, in_=src)

            xqt, wq = [], []
            for h in range(NT):
                t_ = px.tile([128, T], BF16, tag="px", name=f"xqt{h}")
                dma_in(t_, xqt_d[h * 128:(h + 1) * 128, :], h)
                xqt.append(t_)
                w_ = pw.tile([128, H], BF16, tag="pw", name=f"wq{h}")
                dma_in(w_, wqt_d[h * 128:(h + 1) * 128, :], h)
                wq.append(w_)
            xkt, wk = [], []
            for h in range(NT):
                t_ = px.tile([128, T], BF16, tag="px", name=f"xkt{h}")
                dma_in(t_, xkt_d[h * 128:(h + 1) * 128, :], h)
                xkt.append(t_)
                w_ = pw.tile([128, H], BF16, tag="pw", name=f"wk{h}")
                dma_in(w_, wkt_d[h * 128:(h + 1) * 128, :], h)
                wk.append(w_)
            wv = []
            for h in range(NT):
                w_ = pw.tile([128, H], BF16, tag="pw", name=f"wv{h}")
                nc.sync.dma_start(out=w_, in_=wvt_d[h * 128:(h + 1) * 128, :])
                wv.append(w_)
            wo = []
            for h in range(NT):
                w_ = pw.tile([128, H], BF16, tag="pw", name=f"wo{h}")
                nc.sync.dma_start(out=w_, in_=wot_d[h * 128:(h + 1) * 128, :])
                wo.append(w_)
            ident = pms.tile([128, 128], BF16, tag="ms", name="ident")
            nc.sync.dma_start(out=ident, in_=id_d[:, :])

            if with_bq:
                bq_t = pms.tile([128, NT], F32, tag="ms", name="bq_t")
                nc.sync.dma_start(out=bq_t, in_=bq_d.rearrange("(j p) -> p j", p=128))
            if with_bk:
                bk_t = pms.tile([128, NT], F32, tag="ms", name="bk_t")
                nc.sync.dma_start(out=bk_t, in_=bk_d.rearrange("(j p) -> p j", p=128))
            if with_bv:
                # bv broadcast to all (Tq) partitions: [128, H] via DRAM roundtrip
                bv_r = pms.tile([1, H], F32, tag="ms", name="bv_r")
                nc.sync.dma_start(out=bv_r, in_=bv_d.rearrange("h -> 1 h"))
                bv_dr = dscr.tile([1, H], F32, tag="d", name="bv_dr")
                nc.sync.dma_start(out=bv_dr, in_=bv_r)
                bvb = pms.tile([128, H], F32, tag="ms", name="bvb")
                nc.sync.dma_start(out=bvb, in_=bv_dr.partition_broadcast(128))

            # ---- persistent result tiles ----
            qt = [pq.tile([128, T], BF16, tag="pq", name=f"qt{j}") for j in range(PAIRS)]
            kt = [pk.tile([128, T], BF16, tag="pk", name=f"kt{j}") for j in range(PAIRS)]
            vbuf = []
            for i in range(NT):
                vb = pv.tile([128, HEADS * 65], BF16, tag="pv", name=f"vb{i}")
                nc.vector.memset(
                    vb.rearrange("p (g c) -> p g c", c=65)[:, :, 64:65], 1.0)
                vbuf.append(vb)
            # ctx_sb[qb][t]: [128 Tq-sub, H] bf16 (normalized ctx, natural layout)
            ctx_sb = [[pcs.tile([128, H], BF16, tag="cs", name=f"cs{qb}{t}")
                       for t in range(4)] for qb in range(NB)]
            # ctxT[j]: [128 H-dims, T] bf16 (transposed ctx for out-proj)
            ctxT = [pct.tile([128, T], BF16, tag="ct", name=f"ctT{j}")
                    for j in range(PAIRS)]

            # ---- chain generators (each yields after one PE matmul) ----
            def qk_chain(kind, j, qb):
                ps = psF.tile([128, 512], F32, tag="f", name=f"{kind}ch{j}{qb}")
                wsrc = wq if kind == "q" else wk
                xsrc = xqt if kind == "q" else xkt
                for h in range(NT):
                    nc.tensor.matmul(
                        ps, wsrc[h][:, j * 128:(j + 1) * 128],
                        xsrc[h][:, qb * 512:(qb + 1) * 512],
                        start=(h == 0), stop=(h == NT - 1))
                    yield
                dst = (qt if kind == "q" else kt)[j][:, qb * 512:(qb + 1) * 512]
                if kind == "q" and with_bq:
                    nc.vector.tensor_scalar_add(dst, ps, bq_t[:, j:j + 1])
                elif kind == "k" and with_bk:
                    nc.vector.tensor_scalar_add(dst, ps, bk_t[:, j:j + 1])
                else:
                    nc.vector.tensor_copy(out=dst, in_=ps)

            def v_chain(blk, i):
                ps = psF.tile([128, 512], F32, tag="f", name=f"vch{blk}{i}")
                for h in range(NT):
                    nc.tensor.matmul(
                        ps, xkt[h][:, i * 128:(i + 1) * 128],
                        wv[h][:, blk * 512:(blk + 1) * 512],
                        start=(h == 0), stop=(h == NT - 1))
                    yield
                dst = vbuf[i][:, blk * 8 * 65:(blk + 1) * 8 * 65] \
                    .rearrange("p (g c) -> p g c", c=65)[:, :, 0:64]
                nc.vector.tensor_copy(
                    out=dst, in_=ps.rearrange("p (g d) -> p g d", d=64))

            def t_chain(qb, j, pool, ptag):
                # transpose ctx_sb[qb][t][:, j*128:(j+1)*128] -> ctxT[j][:, qb cols]
                ps = pool.tile([128, 512], F32, tag=ptag, name=f"tch{qb}{j}")
                pb = ps.bitcast(BF16)   # [128, 1024] bf16 view
                for t in range(4):
                    nc.tensor.matmul(
                        pb[:, t * 128:(t + 1) * 128],
                        ctx_sb[qb][t][:, j * 128:(j + 1) * 128],
                        ident, start=True, stop=True, is_transpose=True,
                        skip_group_check=True)
                    yield
                nc.vector.tensor_copy(
                    out=ctxT[j][:, qb * 512:(qb + 1) * 512], in_=pb[:, 0:512])

            def o_chain(tt, ob, pool, ptag):
                ps = pool.tile([128, 512], F32, tag=ptag, name=f"och{tt}{ob}")
                for j in range(NT):
                    nc.tensor.matmul(
                        ps, ctxT[j][:, tt * 128:(tt + 1) * 128],
                        wo[j][:, ob * 512:(ob + 1) * 512],
                        start=(j == 0), stop=(j == NT - 1))
                    yield
                o_ = pou.tile([128, 512], F32, tag="ou", name=f"ot{tt}{ob}")
                nc.vector.tensor_copy(out=o_, in_=ps)
                nc.sync.dma_start(
                    out=out_d[tt * 128:(tt + 1) * 128, ob * 512:(ob + 1) * 512],
                    in_=o_)

            fillers = deque()   # items: (tag, generator)
            done_tags = set()

            def pull(n):
                # advance up to n filler matmuls, but stop at a chain boundary:
                # the next chain's first matmul would stall the in-order PE
                # queue on the finished chain's psum eviction.
                k = 0
                while k < n and fillers:
                    try:
                        next(fillers[0][1])
                        k += 1
                    except StopIteration:
                        done_tags.add(fillers[0][0])
                        fillers.popleft()
                        break

            def ensure(*tags):
                # force-drain fillers (in order) until all tags are emitted
                need = [t for t in tags if t not in done_tags]
                while need:
                    if not fillers:
                        raise AssertionError(f"filler tags missing: {need}")
                    tag, gen = fillers[0]
                    for _ in gen:
                        pass
                    done_tags.add(tag)
                    fillers.popleft()
                    need = [t for t in tags if t not in done_tags]

            def run_now(gen):
                for _ in gen:
                    pass

            # ---- pre-attention: 8-slot h-outer waves (all PSUM banks idle
            # here, so borrow psS+psC+psF for full DMA/compute pipelining) ----
            def wave(kind, wname):
                big = [psS.tile([128, 1024], F32, tag="s", name=f"w{wname}{c}")
                       for c in range(2)]
                small = [psC.tile([128, 512], F32, tag="c", name=f"w{wname}c{c}")
                         for c in range(3)]
                small.append(psF.tile([128, 512], F32, tag="f", name=f"w{wname}f"))
                slots = [big[0][:, 0:512], big[0][:, 512:1024],
                         big[1][:, 0:512], big[1][:, 512:1024]] + small
                if kind == "v":
                    # slot s = Tk-tile i=s, H-block 0 (pairs 0-3)
                    for h in range(NT):
                        for s in range(8):
                            nc.tensor.matmul(
                                slots[s], xkt[h][:, s * 128:(s + 1) * 128],
                                wv[h][:, 0:512],
                                start=(h == 0), stop=(h == NT - 1))
                    for s in range(8):
                        dst = vbuf[s][:, 0:8 * 65] \
                            .rearrange("p (g c) -> p g c", c=65)[:, :, 0:64]
                        nc.vector.tensor_copy(
                            out=dst, in_=slots[s].rearrange("p (g d) -> p g d", d=64))
                else:
                    wsrc = wq if kind == "q" else wk
                    xsrc = xqt if kind == "q" else xkt
                    chains = [(j, qb) for j in range(4) for qb in range(NB)]
                    for h in range(NT):
                        for s, (j, qb) in enumerate(chains):
                            nc.tensor.matmul(
                                slots[s], wsrc[h][:, j * 128:(j + 1) * 128],
                                xsrc[h][:, qb * 512:(qb + 1) * 512],
                                start=(h == 0), stop=(h == NT - 1))
                    for s, (j, qb) in enumerate(chains):
                        dst = (qt if kind == "q" else kt)[j][:, qb * 512:(qb + 1) * 512]
                        if kind == "q" and with_bq:
                            nc.vector.tensor_scalar_add(dst, slots[s], bq_t[:, j:j + 1])
                        elif kind == "k" and with_bk:
                            nc.vector.tensor_scalar_add(dst, slots[s], bk_t[:, j:j + 1])
                        else:
                            nc.vector.tensor_copy(out=dst, in_=slots[s])

            wave("q", "q")
            wave("k", "k")

            # remaining projections become fillers inside the attention loop;
            # V block 0 leads (the first unit's lagged ctx consumes it)
            for i in range(NT):
                fillers.append((f"v0.{i}", v_chain(0, i)))
            for qb in range(NB):
                fillers.append((f"q4{qb}", qk_chain("q", 4, qb)))
                fillers.append((f"k4{qb}", qk_chain("k", 4, qb)))
            for i in range(NT):
                fillers.append((f"v1.{i}", v_chain(1, i)))
            for j in (5, 6, 7):
                fillers.append((f"q{j}0", qk_chain("q", j, 0)))
                fillers.append((f"k{j}0", qk_chain("k", j, 0)))
                fillers.append((f"k{j}1", qk_chain("k", j, 1)))
            # qb1 halves of Q for pairs 5-7 are only needed by qb1 attention:
            # defer them (pushed after qb0's transpose/out-proj fillers below)
            deferred_q = [(f"q{j}1", qk_chain("q", j, 1)) for j in (5, 6, 7)]

            # ---- attention ----
            def emit_ctx(j, qb, i, ex, cH):
                for h in range(2):
                    g = 2 * j + h
                    for t in range(4):
                        nc.tensor.matmul(
                            cH[h][:, t * 65:t * 65 + 65],
                            ex[:, h * 512 + t * 128:h * 512 + (t + 1) * 128],
                            vbuf[i][:, g * 65:(g + 1) * 65],
                            start=(i == 0 and t == 0),
                            stop=(i == NT - 1 and t == 0),
                            skip_group_check=(t > 0))

            def normalize(j, qb, cH):
                for h in range(2):
                    g = 2 * j + h
                    rec = prc.tile([128, 4], F32, tag="rc", name=f"rc{qb}{j}{h}")
                    nc.vector.reciprocal(
                        out=rec.rearrange("p (t c) -> p t c", c=1),
                        in_=cH[h][:, 64:64 + 4 * 65]
                        .rearrange("p (t c) -> p t c", c=65)[:, :, 0:1])
                    for t in range(4):
                        dst = ctx_sb[qb][t][:, g * 64:(g + 1) * 64]
                        src = cH[h][:, t * 65:t * 65 + 64]
                        if with_bv:
                            nc.vector.scalar_tensor_tensor(
                                out=dst, in0=src, scalar=rec[:, t:t + 1],
                                in1=bvb[:, g * 64:(g + 1) * 64],
                                op0=mybir.AluOpType.mult,
                                op1=mybir.AluOpType.add)
                        else:
                            nc.vector.tensor_scalar_mul(dst, src, rec[:, t:t + 1])

            for qb in range(NB):
                for j in range(PAIRS):
                    if j >= 4:
                        ensure(f"q{j}{qb}", f"k{j}0", f"k{j}1",
                               *(f"v1.{i}" for i in range(NT)))
                    cH = [psC.tile([128, 512], F32, tag="c", name=f"c{qb}{j}{h}")
                          for h in range(2)]
                    exs = []
                    blk = j // 4
                    # the very first unit lags ctx by 4 so V block 0 (a filler
                    # chain here, not a pre-attention wave) has time to land
                    lag = 4 if (qb == 0 and j == 0) else 1
                    for i in range(NT):
                        pull(3)
                        ss = psS.tile([128, 1024], F32, tag="s", name=f"ss{qb}{j}{i}")
                        for h in range(2):
                            r0 = 64 * h
                            nc.tensor.matmul(
                                ss[:, h * 512:(h + 1) * 512],
                                kt[j][r0:r0 + 64, i * 128:(i + 1) * 128],
                                qt[j][r0:r0 + 64, qb * 512:(qb + 1) * 512],
                                start=True, stop=True)
                        ex = pex.tile([128, 1024], BF16, tag="e", name=f"ex{qb}{j}{i}")
                        nc.scalar.activation(out=ex, in_=ss, func=AF.Exp)
                        exs.append(ex)
                        if i >= lag:
                            ensure(f"v{blk}.{i - lag}")
                            emit_ctx(j, qb, i - lag, exs[i - lag], cH)
                    for i in range(NT - lag, NT):
                        ensure(f"v{blk}.{i}")
                        emit_ctx(j, qb, i, exs[i], cH)
                    normalize(j, qb, cH)
                    # this pair's ctx_sb columns are final: transpose chain
                    # becomes a filler right away
                    fillers.append((f"t{qb}{j}", t_chain(qb, j, psF, "f")))
                if qb == 0:
                    # deferred Q halves first: ensure() at (qb1, j>=5) must not
                    # force-drain the out-proj chains behind them
                    fillers.extend(deferred_q)
                # out-proj needs ctxT for all pairs (qb0's chains fill qb1's
                # attention on psF; qb1's run at the tail on psC's free banks)
                pool, ptag = (psF, "f") if qb == 0 else (psC, "c")
                for tt in range(qb * 4, (qb + 1) * 4):
                    for ob in range(NB):
                        fillers.append((f"o{tt}{ob}", o_chain(tt, ob, pool, ptag)))

            while fillers:
                pull(64)

    nc.finalize()
    return nc


def _prepare(inputs_q, inputs_kv, w_q, b_q, w_kv, b_kv, w_o, b_o,
             r_q, s_q, r_kv, s_kv, heads):
    inputs_q = np.asarray(inputs_q, np.float32)
    inputs_kv = np.asarray(inputs_kv, np.float32)
    w_q = np.asarray(w_q, np.float32)
    b_q = np.asarray(b_q, np.float32)
    w_kv = np.asarray(w_kv, np.float32)
    b_kv = np.asarray(b_kv, np.float32)
    w_o = np.asarray(w_o, np.float32)
    b_o = np.asarray(b_o, np.float32)
    r_q = np.asarray(r_q, np.float32)
    s_q = np.asarray(s_q, np.float32)
    r_kv = np.asarray(r_kv, np.float32)
    s_kv = np.asarray(s_kv, np.float32)
    heads = int(heads)
    assert heads == HEADS and inputs_q.shape == (T, B, H)

    scale = np.float32((H // heads) ** -0.5)

    # split w_kv / b_kv / s_kv into K and V parts (2H axis = heads x {k,v} x hd)
    w_kv_r = w_kv.reshape(HEADS, 2, HD, H)
    k_w = w_kv_r[:, 0].reshape(H, H)
    v_w = w_kv_r[:, 1].reshape(H, H)
    b_kv_r = b_kv.reshape(HEADS, 2, HD)
    bk = np.ascontiguousarray(b_kv_r[:, 0].reshape(H))
    bv = np.ascontiguousarray(b_kv_r[:, 1].reshape(H))
    s_kv_r = s_kv.reshape(B, HEADS, 2, HD)
    s_k = s_kv_r[:, :, 0].reshape(B, H)
    s_v = s_kv_r[:, :, 1].reshape(B, H)

    with_bq = bool(np.any(b_q))
    with_bk = bool(np.any(bk))
    with_bv = bool(np.any(bv))
    key = (with_bq, with_bk, with_bv)

    wot = np.ascontiguousarray(w_o.T).astype(NPBF)
    ident = np.eye(128, dtype=NPBF)
    in_maps = []
    for b in range(B):
        m = {
            "xqt": np.ascontiguousarray(inputs_q[:, b, :].T).astype(NPBF),
            "xkt": np.ascontiguousarray(inputs_kv[:, b, :].T).astype(NPBF),
            # W''[o,h] = s[o]*W[o,h]*r[h]; lhsT wants [h, o] = W''.T
            "wqt": np.ascontiguousarray(
                (w_q * (s_q[b] * scale)[:, None] * r_q[b][None, :]).T).astype(NPBF),
            "wkt": np.ascontiguousarray(
                (k_w * s_k[b][:, None] * r_kv[b][None, :]).T).astype(NPBF),
            "wvt": np.ascontiguousarray(
                (v_w * s_v[b][:, None] * r_kv[b][None, :]).T).astype(NPBF),
            "wot": wot,
            "ident": ident,
        }
        if with_bq:
            m["bq"] = b_q * scale
        if with_bk:
            m["bk"] = bk
        if with_bv:
            m["bv"] = bv
        in_maps.append(m)
    return key, in_maps


def kernel(inputs_q, inputs_kv, w_q, b_q, w_kv, b_kv, w_o, b_o,
           r_q, s_q, r_kv, s_kv, heads):
    b_o = np.asarray(b_o, np.float32)
    key, in_maps = _prepare(inputs_q, inputs_kv, w_q, b_q, w_kv, b_kv,
                            w_o, b_o, r_q, s_q, r_kv, s_kv, heads)
    if key not in _cache:
        _cache[key] = _build(*key)
    nc = _cache[key]

    global _last_in_maps
    _last_in_maps = in_maps
    res = run_bass_kernel_spmd(nc, in_maps, list(range(B)))
    out = np.empty((T, B, H), np.float32)
    for b in range(B):
        out[:, b, :] = res.results[b]["out"]
    out += b_o
    return out


# revision 31
# speedup vs baseline: 1.3306x; 1.3306x over previous
"""Trainium2 Bass kernel for BatchEnsemble encoder-decoder multihead attention.

Problem (hardcoded shapes): Tq=Tk=1024, B=8, H=1024, heads=16, hd=64.

Sharding: pure data parallelism - batch B=8 across the 8 NeuronCores, one
batch element per core. No collectives.

Per-core math (batch b), BatchEnsemble rank-1 factors and the 1/sqrt(hd)
scale folded into per-core bf16 weights on the host:
    Q^T = Wq''^T.T @ Xq^T          [H, Tq]  (head-pair dims on partitions)
    K^T = Wk''^T.T @ Xk^T          [H, Tk]
    V   = Xk^T.T @ Wv''^T          [Tk, H]  (natural layout, [V|1] stationary)
    per head g:
      S^T = K_g^T.T @ Q_g^T        [Tk, Tq]   bf16 matmuls
      E   = exp(S^T) -> bf16       (scores bounded, no max-subtraction)
      [ctx | den] = E_tile.T @ [V_g | 1]   [Tq-tile, 65]  <- transposed ctx:
            stationary = E tile (128 Tk x 128 Tq), moving = [V|1]: full PE
            utilization (the ones column gives the softmax denominator).
      ctx = ctx * (1/den)          (DVE per-partition scalar, no broadcast DMA)
    ctx^T via PE transpose (identity RHS, bf16)  -> [H, Tq]
    out  = ctx^T.T @ Wo^T          [Tq, H]

All matmul operands bf16 (1 cycle/row on the PE, fp32 PSUM accumulate);
measured end-to-end absmax error ~5e-3 of output scale.

Schedule: the attention inner loop is ACT(exp)-paced, so projection /
out-projection / transpose matmuls are emitted as fine-grained "fillers"
(a few matmuls per attention i-step) to keep the PE busy during exp waits.
ctx accumulators for 4 Tq-subtiles share one PSUM bank (regions at 65-col
offsets, lazy-zero semantics with skip_group_check on regions 1-3).
Out-projection for the first Tq half runs during the second half's
attention.
"""

from collections import deque

import numpy as np
import ml_dtypes

import concourse.bass as bass
import concourse.tile as tile
import concourse.mybir as mybir
from concourse import bacc
from concourse.bass_utils import run_bass_kernel_spmd

F32 = mybir.dt.float32
BF16 = mybir.dt.bfloat16
AF = mybir.ActivationFunctionType
NPBF = ml_dtypes.bfloat16

T = 1024        # Tq = Tk
H = 1024
B = 8
HEADS = 16
HD = 64
NT = T // 128   # 8 x 128-tiles
NB = T // 512   # 2 x 512-blocks (qb)
PAIRS = HEADS // 2

_cache = {}
_last_in_maps = None


def _build(with_bq, with_bk, with_bv):
    nc = bacc.Bacc("TRN2", target_bir_lowering=False, debug=False)

    xqt_d = nc.dram_tensor("xqt", [H, T], BF16, kind="ExternalInput")
    xkt_d = nc.dram_tensor("xkt", [H, T], BF16, kind="ExternalInput")
    wqt_d = nc.dram_tensor("wqt", [H, H], BF16, kind="ExternalInput")
    wkt_d = nc.dram_tensor("wkt", [H, H], BF16, kind="ExternalInput")
    wvt_d = nc.dram_tensor("wvt", [H, H], BF16, kind="ExternalInput")
    wot_d = nc.dram_tensor("wot", [H, H], BF16, kind="ExternalInput")
    id_d = nc.dram_tensor("ident", [128, 128], BF16, kind="ExternalInput")
    bq_d = nc.dram_tensor("bq", [H], F32, kind="ExternalInput") if with_bq else None
    bk_d = nc.dram_tensor("bk", [H], F32, kind="ExternalInput") if with_bk else None
    bv_d = nc.dram_tensor("bv", [H], F32, kind="ExternalInput") if with_bv else None
    out_d = nc.dram_tensor("out", [T, H], F32, kind="ExternalOutput")

    with tile.TileContext(nc) as tc:
        with tc.tile_pool(name="px", bufs=16) as px, \
             tc.tile_pool(name="pw", bufs=32) as pw, \
             tc.tile_pool(name="pq", bufs=8) as pq, \
             tc.tile_pool(name="pk", bufs=8) as pk, \
             tc.tile_pool(name="pv", bufs=8) as pv, \
             tc.tile_pool(name="pex", bufs=6) as pex, \
             tc.tile_pool(name="pcs", bufs=8) as pcs, \
             tc.tile_pool(name="pct", bufs=8) as pct, \
             tc.tile_pool(name="pou", bufs=3) as pou, \
             tc.tile_pool(name="prc", bufs=6) as prc, \
             tc.tile_pool(name="pms", bufs=4) as pms, \
             tc.tile_pool(name="dscr", bufs=2, space="DRAM") as dscr, \
             tc.tile_pool(name="psS", bufs=2, space="PSUM") as psS, \
             tc.tile_pool(name="psC", bufs=3, space="PSUM") as psC, \
             tc.tile_pool(name="psF", bufs=1, space="PSUM") as psF:

            # ---- input DMAs, consumption order ----
            def dma_in(tile_, src, h):
                # split the first h-tile into column halves so the first wave
                # matmul can start on half the bytes
                if h == 0:
                    nc.sync.dma_start(out=tile_[:, 0:512], in_=src[:, 0:512])
                    nc.sync.dma_start(out=tile_[:, 512:1024], in_=src[:, 512:1024])
                else:
                    nc.sync.dma_start(out=tile_, in_=src)

            xqt, wq = [], []
            for h in range(NT):
                t_ = px.tile([128, T], BF16, tag="px", name=f"xqt{h}")
                dma_in(t_, xqt_d[h * 128:(h + 1) * 128, :], h)
                xqt.append(t_)
                w_ = pw.tile([128, H], BF16, tag="pw", name=f"wq{h}")
                dma_in(w_, wqt_d[h * 128:(h + 1) * 128, :], h)
                wq.append(w_)
            xkt, wk = [], []
            for h in range(NT):
                t_ = px.tile([128, T], BF16, tag="px", name=f"xkt{h}")
                dma_in(t_, xkt_d[h * 128:(h + 1) * 128, :], h)
                xkt.append(t_)
                w_ = pw.tile([128, H], BF16, tag="pw", name=f"wk{h}")
                dma_in(w_, wkt_d[h * 128:(h + 1) * 128, :], h)
                wk.append(w_)
            wv = []
            for h in range(NT):
                w_ = pw.tile([128, H], BF16, tag="pw", name=f"wv{h}")
                nc.sync.dma_start(out=w_, in_=wvt_d[h * 128:(h + 1) * 128, :])
                wv.append(w_)
            wo = []
            for h in range(NT):
                w_ = pw.tile([128, H], BF16, tag="pw", name=f"wo{h}")
                nc.sync.dma_start(out=w_, in_=wot_d[h * 128:(h + 1) * 128, :])
                wo.append(w_)
            ident = pms.tile([128, 128], BF16, tag="ms", name="ident")
            nc.sync.dma_start(out=ident, in_=id_d[:, :])

            if with_bq:
                bq_t = pms.tile([128, NT], F32, tag="ms", name="bq_t")
                nc.sync.dma_start(out=bq_t, in_=bq_d.rearrange("(j p) -> p j", p=128))
            if with_bk:
                bk_t = pms.tile([128, NT], F32, tag="ms", name="bk_t")
                nc.sync.dma_start(out=bk_t, in_=bk_d.rearrange("(j p) -> p j", p=128))
            if with_bv:
                # bv broadcast to all (Tq) partitions: [128, H] via DRAM roundtrip
                bv_r = pms.tile([1, H], F32, tag="ms", name="bv_r")
                nc.sync.dma_start(out=bv_r, in_=bv_d.rearrange("h -> 1 h"))
                bv_dr = dscr.tile([1, H], F32, tag="d", name="bv_dr")
                nc.sync.dma_start(out=bv_dr, in_=bv_r)
                bvb = pms.tile([128, H], F32, tag="ms", name="bvb")
                nc.sync.dma_start(out=bvb, in_=bv_dr.partition_broadcast(128))

            # ---- persistent result tiles ----
            qt = [pq.tile([128, T], BF16, tag="pq", name=f"qt{j}") for j in range(PAIRS)]
            kt = [pk.tile([128, T], BF16, tag="pk", name=f"kt{j}") for j in range(PAIRS)]
            vbuf = []
            for i in range(NT):
                vb = pv.tile([128, HEADS * 65], BF16, tag="pv", name=f"vb{i}")
                nc.vector.memset(
                    vb.rearrange("p (g c) -> p g c", c=65)[:, :, 64:65], 1.0)
                vbuf.append(vb)
            # ctx_sb[qb][t]: [128 Tq-sub, H] bf16 (normalized ctx, natural layout)
            ctx_sb = [[pcs.tile([128, H], BF16, tag="cs", name=f"cs{qb}{t}")
                       for t in range(4)] for qb in range(NB)]
            # ctxT[j]: [128 H-dims, T] bf16 (transposed ctx for out-proj)
            ctxT = [pct.tile([128, T], BF16, tag="ct", name=f"ctT{j}")
                    for j in range(PAIRS)]

            # ---- chain generators (each yields after one PE matmul) ----
            def qk_chain(kind, j, qb):
                ps = psF.tile([128, 512], F32, tag="f", name=f"{kind}ch{j}{qb}")
                wsrc = wq if kind == "q" else wk
                xsrc = xqt if kind == "q" else xkt
                for h in range(NT):
                    nc.tensor.matmul(
                        ps, wsrc[h][:, j * 128:(j + 1) * 128],
                        xsrc[h][:, qb * 512:(qb + 1) * 512],
                        start=(h == 0), stop=(h == NT - 1))
                    yield
                dst = (qt if kind == "q" else kt)[j][:, qb * 512:(qb + 1) * 512]
                if kind == "q" and with_bq:
                    nc.vector.tensor_scalar_add(dst, ps, bq_t[:, j:j + 1])
                elif kind == "k" and with_bk:
                    nc.vector.tensor_scalar_add(dst, ps, bk_t[:, j:j + 1])
                else:
                    nc.vector.tensor_copy(out=dst, in_=ps)

            def v_chain(blk, i):
                ps = psF.tile([128, 512], F32, tag="f", name=f"vch{blk}{i}")
                for h in range(NT):
                    nc.tensor.matmul(
                        ps, xkt[h][:, i * 128:(i + 1) * 128],
                        wv[h][:, blk * 512:(blk + 1) * 512],
                        start=(h == 0), stop=(h == NT - 1))
                    yield
                dst = vbuf[i][:, blk * 8 * 65:(blk + 1) * 8 * 65] \
                    .rearrange("p (g c) -> p g c", c=65)[:, :, 0:64]
                nc.vector.tensor_copy(
                    out=dst, in_=ps.rearrange("p (g d) -> p g d", d=64))

            def t_chain(qb, j, pool, ptag):
                # transpose ctx_sb[qb][t][:, j*128:(j+1)*128] -> ctxT[j][:, qb cols]
                ps = pool.tile([128, 512], F32, tag=ptag, name=f"tch{qb}{j}")
                pb = ps.bitcast(BF16)   # [128, 1024] bf16 view
                for t in range(4):
                    nc.tensor.matmul(
                        pb[:, t * 128:(t + 1) * 128],
                        ctx_sb[qb][t][:, j * 128:(j + 1) * 128],
                        ident, start=True, stop=True, is_transpose=True,
                        skip_group_check=True)
                    yield
                nc.vector.tensor_copy(
                    out=ctxT[j][:, qb * 512:(qb + 1) * 512], in_=pb[:, 0:512])

            def o_chain(tt, ob, pool, ptag):
                ps = pool.tile([128, 512], F32, tag=ptag, name=f"och{tt}{ob}")
                for j in range(NT):
                    nc.tensor.matmul(
                        ps, ctxT[j][:, tt * 128:(tt + 1) * 128],
                        wo[j][:, ob * 512:(ob + 1) * 512],
                        start=(j == 0), stop=(j == NT - 1))
                    yield
                o_ = pou.tile([128, 512], F32, tag="ou", name=f"ot{tt}{ob}")
                nc.vector.tensor_copy(out=o_, in_=ps)
                nc.sync.dma_start(
                    out=out_d[tt * 128:(tt + 1) * 128, ob * 512:(ob + 1) * 512],
                    in_=o_)

            fillers = deque()   # items: (tag, generator)
            done_tags = set()

            def pull(n):
                # advance up to n filler matmuls, but stop at a chain boundary:
                # the next chain's first matmul would stall the in-order PE
                # queue on the finished chain's psum eviction.
                k = 0
                while k < n and fillers:
                    try:
                        next(fillers[0][1])
                        k += 1
                    except StopIteration:
                        done_tags.add(fillers[0][0])
                        fillers.popleft()
                        break

            def ensure(*tags):
                # force-drain fillers (in order) until all tags are emitted
                need = [t for t in tags if t not in done_tags]
                while need:
                    if not fillers:
                        raise AssertionError(f"filler tags missing: {need}")
                    tag, gen = fillers[0]
                    for _ in gen:
                        pass
                    done_tags.add(tag)
                    fillers.popleft()
                    need = [t for t in tags if t not in done_tags]

            def run_now(gen):
                for _ in gen:
                    pass

            # ---- pre-attention: 8-slot h-outer waves (all PSUM banks idle
            # here, so borrow psS+psC+psF for full DMA/compute pipelining) ----
            def wave(kind, wname):
                big = [psS.tile([128, 1024], F32, tag="s", name=f"w{wname}{c}")
                       for c in range(2)]
                small = [psC.tile([128, 512], F32, tag="c", name=f"w{wname}c{c}")
                         for c in range(3)]
                small.append(psF.tile([128, 512], F32, tag="f", name=f"w{wname}f"))
                slots = [big[0][:, 0:512], big[0][:, 512:1024],
                         big[1][:, 0:512], big[1][:, 512:1024]] + small
                if kind == "v":
                    # slot s = Tk-tile i=s, H-block 0 (pairs 0-3)
                    for h in range(NT):
                        for s in range(8):
                            nc.tensor.matmul(
                                slots[s], xkt[h][:, s * 128:(s + 1) * 128],
                                wv[h][:, 0:512],
                                start=(h == 0), stop=(h == NT - 1))
                    for s in range(8):
                        dst = vbuf[s][:, 0:8 * 65] \
                            .rearrange("p (g c) -> p g c", c=65)[:, :, 0:64]
                        nc.vector.tensor_copy(
                            out=dst, in_=slots[s].rearrange("p (g d) -> p g d", d=64))
                else:
                    wsrc = wq if kind == "q" else wk
                    xsrc = xqt if kind == "q" else xkt
                    chains = [(j, qb) for j in range(4) for qb in range(NB)]
                    for h in range(NT):
                        for s, (j, qb) in enumerate(chains):
                            nc.tensor.matmul(
                                slots[s], wsrc[h][:, j * 128:(j + 1) * 128],
                                xsrc[h][:, qb * 512:(qb + 1) * 512],
                                start=(h == 0), stop=(h == NT - 1))
                    for s, (j, qb) in enumerate(chains):
                        dst = (qt if kind == "q" else kt)[j][:, qb * 512:(qb + 1) * 512]
                        if kind == "q" and with_bq:
                            nc.vector.tensor_scalar_add(dst, slots[s], bq_t[:, j:j + 1])
                        elif kind == "k" and with_bk:
                            nc.vector.tensor_scalar_add(dst, slots[s], bk_t[:, j:j + 1])
                        else:
                            nc.vector.tensor_copy(out=dst, in_=slots[s])

            wave("q", "q")
            wave("k", "k")

            # remaining projections become fillers inside the attention loop;
            # V block 0 leads (the first unit's lagged ctx consumes it)
            for i in range(NT):
                fillers.append((f"v0.{i}", v_chain(0, i)))
            for qb in range(NB):
                fillers.append((f"q4{qb}", qk_chain("q", 4, qb)))
                fillers.append((f"k4{qb}", qk_chain("k", 4, qb)))
            for i in range(NT):
                fillers.append((f"v1.{i}", v_chain(1, i)))
            for j in (5, 6, 7):
                fillers.append((f"q{j}0", qk_chain("q", j, 0)))
                fillers.append((f"k{j}0", qk_chain("k", j, 0)))
                fillers.append((f"k{j}1", qk_chain("k", j, 1)))
            # qb1 halves of Q for pairs 5-7 are only needed by qb1 attention:
            # defer them (pushed after qb0's transpose/out-proj fillers below)
            deferred_q = [(f"q{j}1", qk_chain("q", j, 1)) for j in (5, 6, 7)]

            # ---- attention ----
            def emit_ctx(j, qb, i, ex, cH):
                for h in range(2):
                    g = 2 * j + h
                    for t in range(4):
                        nc.tensor.matmul(
                            cH[h][:, t * 65:t * 65 + 65],
                            ex[:, h * 512 + t * 128:h * 512 + (t + 1) * 128],
                            vbuf[i][:, g * 65:(g + 1) * 65],
                            start=(i == 0 and t == 0),
                            stop=(i == NT - 1 and t == 0),
                            skip_group_check=(t > 0))

            def normalize(j, qb, cH):
                for h in range(2):
                    g = 2 * j + h
                    rec = prc.tile([128, 4], F32, tag="rc", name=f"rc{qb}{j}{h}")
                    nc.vector.reciprocal(
                        out=rec.rearrange("p (t c) -> p t c", c=1),
                        in_=cH[h][:, 64:64 + 4 * 65]
                        .rearrange("p (t c) -> p t c", c=65)[:, :, 0:1])
                    for t in range(4):
                        dst = ctx_sb[qb][t][:, g * 64:(g + 1) * 64]
                        src = cH[h][:, t * 65:t * 65 + 64]
                        if with_bv:
                            nc.vector.scalar_tensor_tensor(
                                out=dst, in0=src, scalar=rec[:, t:t + 1],
                                in1=bvb[:, g * 64:(g + 1) * 64],
                                op0=mybir.AluOpType.mult,
                                op1=mybir.AluOpType.add)
                        else:
                            nc.vector.tensor_scalar_mul(dst, src, rec[:, t:t + 1])

            for qb in range(NB):
                for j in range(PAIRS):
                    if j >= 4:
                        ensure(f"q{j}{qb}", f"k{j}0", f"k{j}1",
                               *(f"v1.{i}" for i in range(NT)))
                    cH = [psC.tile([128, 512], F32, tag="c", name=f"c{qb}{j}{h}")
                          for h in range(2)]
                    exs = []
                    blk = j // 4
                    # the very first unit lags ctx by 4 so V block 0 (a filler
                    # chain here, not a pre-attention wave) has time to land
                    lag = 4 if (qb == 0 and j == 0) else 1
                    for i in range(NT):
                        # scores+exp first: a stalled filler matmul must never
                        # sit between the exp and the scores feeding it (ACT
                        # is the attention pacer); fillers absorb the slack
                        # after the exp is dispatched.
                        ss = psS.tile([128, 1024], F32, tag="s", name=f"ss{qb}{j}{i}")
                        for h in range(2):
                            r0 = 64 * h
                            nc.tensor.matmul(
                                ss[:, h * 512:(h + 1) * 512],
                                kt[j][r0:r0 + 64, i * 128:(i + 1) * 128],
                                qt[j][r0:r0 + 64, qb * 512:(qb + 1) * 512],
                                start=True, stop=True)
                        ex = pex.tile([128, 1024], BF16, tag="e", name=f"ex{qb}{j}{i}")
                        nc.scalar.activation(out=ex, in_=ss, func=AF.Exp)
                        exs.append(ex)
                        if i >= lag:
                            ensure(f"v{blk}.{i - lag}")
                            emit_ctx(j, qb, i - lag, exs[i - lag], cH)
                        pull(3)
                    for i in range(NT - lag, NT):
                        ensure(f"v{blk}.{i}")
                        emit_ctx(j, qb, i, exs[i], cH)
                    normalize(j, qb, cH)
                    # this pair's ctx_sb columns are final: transpose chain
                    # becomes a filler right away
                    fillers.append((f"t{qb}{j}", t_chain(qb, j, psF, "f")))
                if qb == 0:
                    # deferred Q halves first: ensure() at (qb1, j>=5) must not
                    # force-drain the out-proj chains behind them
                    fillers.extend(deferred_q)
                # out-proj needs ctxT for all pairs (qb0's chains fill qb1's
                # attention on psF; qb1's run at the tail on psC's free banks)
                pool, ptag = (psF, "f") if qb == 0 else (psC, "c")
                for tt in range(qb * 4, (qb + 1) * 4):
                    for ob in range(NB):
                        fillers.append((f"o{tt}{ob}", o_chain(tt, ob, pool, ptag)))

            while fillers:
                pull(64)

    nc.finalize()
    return nc


def _prepare(inputs_q, inputs_kv, w_q, b_q, w_kv, b_kv, w_o, b_o,
             r_q, s_q, r_kv, s_kv, heads):
    inputs_q = np.asarray(inputs_q, np.float32)
    inputs_kv = np.asarray(inputs_kv, np.float32)
    w_q = np.asarray(w_q, np.float32)
    b_q = np.asarray(b_q, np.float32)
    w_kv = np.asarray(w_kv, np.float32)
    b_kv = np.asarray(b_kv, np.float32)
    w_o = np.asarray(w_o, np.float32)
    b_o = np.asarray(b_o, np.float32)
    r_q = np.asarray(r_q, np.float32)
    s_q = np.asarray(s_q, np.float32)
    r_kv = np.asarray(r_kv, np.float32)
    s_kv = np.asarray(s_kv, np.float32)
    heads = int(heads)
    assert heads == HEADS and inputs_q.shape == (T, B, H)

    scale = np.float32((H // heads) ** -0.5)

    # split w_kv / b_kv / s_kv into K and V parts (2H axis = heads x {k,v} x hd)
    w_kv_r = w_kv.reshape(HEADS, 2, HD, H)
    k_w = w_kv_r[:, 0].reshape(H, H)
    v_w = w_kv_r[:, 1].reshape(H, H)
    b_kv_r = b_kv.reshape(HEADS, 2, HD)
    bk = np.ascontiguousarray(b_kv_r[:, 0].reshape(H))
    bv = np.ascontiguousarray(b_kv_r[:, 1].reshape(H))
    s_kv_r = s_kv.reshape(B, HEADS, 2, HD)
    s_k = s_kv_r[:, :, 0].reshape(B, H)
    s_v = s_kv_r[:, :, 1].reshape(B, H)

    with_bq = bool(np.any(b_q))
    with_bk = bool(np.any(bk))
    with_bv = bool(np.any(bv))
    key = (with_bq, with_bk, with_bv)

    wot = np.ascontiguousarray(w_o.T).astype(NPBF)
    ident = np.eye(128, dtype=NPBF)
    in_maps = []
    for b in range(B):
        m = {
            "xqt": np.ascontiguousarray(inputs_q[:, b, :].T).astype(NPBF),
            "xkt": np.ascontiguousarray(inputs_kv[:, b, :].T).astype(NPBF),
            # W''[o,h] = s[o]*W[o,h]*r[h]; lhsT wants [h, o] = W''.T
            "wqt": np.ascontiguousarray(
                (w_q * (s_q[b] * scale)[:, None] * r_q[b][None, :]).T).astype(NPBF),
            "wkt": np.ascontiguousarray(
                (k_w * s_k[b][:, None] * r_kv[b][None, :]).T).astype(NPBF),
            "wvt": np.ascontiguousarray(
                (v_w * s_v[b][:, None] * r_kv[b][None, :]).T).astype(NPBF),
            "wot": wot,
            "ident": ident,
        }
        if with_bq:
            m["bq"] = b_q * scale
        if with_bk:
            m["bk"] = bk
        if with_bv:
            m["bv"] = bv
        in_maps.append(m)
    return key, in_maps


def kernel(inputs_q, inputs_kv, w_q, b_q, w_kv, b_kv, w_o, b_o,
           r_q, s_q, r_kv, s_kv, heads):
    b_o = np.asarray(b_o, np.float32)
    key, in_maps = _prepare(inputs_q, inputs_kv, w_q, b_q, w_kv, b_kv,
                            w_o, b_o, r_q, s_q, r_kv, s_kv, heads)
    if key not in _cache:
        _cache[key] = _build(*key)
    nc = _cache[key]

    global _last_in_maps
    _last_in_maps = in_maps
    res = run_bass_kernel_spmd(nc, in_maps, list(range(B)))
    out = np.empty((T, B, H), np.float32)
    for b in range(B):
        out[:, b, :] = res.results[b]["out"]
    out += b_o
    return out


# revision 39
# speedup vs baseline: 1.3313x; 1.0005x over previous
"""Trainium2 Bass kernel for BatchEnsemble encoder-decoder multihead attention.

Problem (hardcoded shapes): Tq=Tk=1024, B=8, H=1024, heads=16, hd=64.

Sharding: pure data parallelism - batch B=8 across the 8 NeuronCores, one
batch element per core. No collectives.

Per-core math (batch b), BatchEnsemble rank-1 factors and the 1/sqrt(hd)
scale folded into per-core bf16 weights on the host:
    Q^T = Wq''^T.T @ Xq^T          [H, Tq]  (head-pair dims on partitions)
    K^T = Wk''^T.T @ Xk^T          [H, Tk]
    V   = Xk^T.T @ Wv''^T          [Tk, H]  (natural layout, [V|1] stationary)
    per head g:
      S^T = K_g^T.T @ Q_g^T        [Tk, Tq]   bf16 matmuls
      E   = exp(S^T) -> bf16       (scores bounded, no max-subtraction)
      [ctx | den] = E_tile.T @ [V_g | 1]   [Tq-tile, 65]  <- transposed ctx:
            stationary = E tile (128 Tk x 128 Tq), moving = [V|1]: full PE
            utilization (the ones column gives the softmax denominator).
      ctx = ctx * (1/den)          (DVE per-partition scalar, no broadcast DMA)
    ctx^T via PE transpose (identity RHS, bf16)  -> [H, Tq]
    out  = ctx^T.T @ Wo^T          [Tq, H]

All matmul operands bf16 (1 cycle/row on the PE, fp32 PSUM accumulate);
measured end-to-end absmax error ~5e-3 of output scale.

Schedule: the attention inner loop is ACT(exp)-paced, so projection /
out-projection / transpose matmuls are emitted as fine-grained "fillers"
(a few matmuls per attention i-step) to keep the PE busy during exp waits.
ctx accumulators for 4 Tq-subtiles share one PSUM bank (regions at 65-col
offsets, lazy-zero semantics with skip_group_check on regions 1-3).
Out-projection for the first Tq half runs during the second half's
attention.
"""

from collections import deque

import numpy as np
import ml_dtypes

import concourse.bass as bass
import concourse.tile as tile
import concourse.mybir as mybir
from concourse import bacc
from concourse.bass_utils import run_bass_kernel_spmd

F32 = mybir.dt.float32
BF16 = mybir.dt.bfloat16
AF = mybir.ActivationFunctionType
NPBF = ml_dtypes.bfloat16

T = 1024        # Tq = Tk
H = 1024
B = 8
HEADS = 16
HD = 64
NT = T // 128   # 8 x 128-tiles
NB = T // 512   # 2 x 512-blocks (qb)
PAIRS = HEADS // 2

_cache = {}
_last_in_maps = None


def _build(with_bq, with_bk, with_bv):
    nc = bacc.Bacc("TRN2", target_bir_lowering=False, debug=False)

    xqt_d = nc.dram_tensor("xqt", [H, T], BF16, kind="ExternalInput")
    xkt_d = nc.dram_tensor("xkt", [H, T], BF16, kind="ExternalInput")
    wqt_d = nc.dram_tensor("wqt", [H, H], BF16, kind="ExternalInput")
    wkt_d = nc.dram_tensor("wkt", [H, H], BF16, kind="ExternalInput")
    wvt_d = nc.dram_tensor("wvt", [H, H], BF16, kind="ExternalInput")
    wot_d = nc.dram_tensor("wot", [H, H], BF16, kind="ExternalInput")
    id_d = nc.dram_tensor("ident", [128, 128], BF16, kind="ExternalInput")
    bq_d = nc.dram_tensor("bq", [H], F32, kind="ExternalInput") if with_bq else None
    bk_d = nc.dram_tensor("bk", [H], F32, kind="ExternalInput") if with_bk else None
    bv_d = nc.dram_tensor("bv", [H], F32, kind="ExternalInput") if with_bv else None
    out_d = nc.dram_tensor("out", [T, H], F32, kind="ExternalOutput")

    with tile.TileContext(nc) as tc:
        with tc.tile_pool(name="px", bufs=16) as px, \
             tc.tile_pool(name="pw", bufs=32) as pw, \
             tc.tile_pool(name="pq", bufs=8) as pq, \
             tc.tile_pool(name="pk", bufs=8) as pk, \
             tc.tile_pool(name="pv", bufs=8) as pv, \
             tc.tile_pool(name="pex", bufs=6) as pex, \
             tc.tile_pool(name="pcs", bufs=8) as pcs, \
             tc.tile_pool(name="pct", bufs=8) as pct, \
             tc.tile_pool(name="pou", bufs=3) as pou, \
             tc.tile_pool(name="prc", bufs=6) as prc, \
             tc.tile_pool(name="pms", bufs=4) as pms, \
             tc.tile_pool(name="dscr", bufs=2, space="DRAM") as dscr, \
             tc.tile_pool(name="psS", bufs=2, space="PSUM") as psS, \
             tc.tile_pool(name="psC", bufs=3, space="PSUM") as psC, \
             tc.tile_pool(name="psF", bufs=1, space="PSUM") as psF:

            # ---- input DMAs, consumption order ----
            def dma_in(tile_, src, h):
                # split the first h-tile into column halves so the first wave
                # matmul can start on half the bytes
                if h == 0:
                    nc.sync.dma_start(out=tile_[:, 0:512], in_=src[:, 0:512])
                    nc.sync.dma_start(out=tile_[:, 512:1024], in_=src[:, 512:1024])
                else:
                    nc.sync.dma_start(out=tile_, in_=src)

            xqt, wq = [], []
            for h in range(NT):
                t_ = px.tile([128, T], BF16, tag="px", name=f"xqt{h}")
                dma_in(t_, xqt_d[h * 128:(h + 1) * 128, :], h)
                xqt.append(t_)
                w_ = pw.tile([128, H], BF16, tag="pw", name=f"wq{h}")
                dma_in(w_, wqt_d[h * 128:(h + 1) * 128, :], h)
                wq.append(w_)
            xkt, wk = [], []
            for h in range(NT):
                t_ = px.tile([128, T], BF16, tag="px", name=f"xkt{h}")
                dma_in(t_, xkt_d[h * 128:(h + 1) * 128, :], h)
                xkt.append(t_)
                w_ = pw.tile([128, H], BF16, tag="pw", name=f"wk{h}")
                dma_in(w_, wkt_d[h * 128:(h + 1) * 128, :], h)
                wk.append(w_)
            wv = []
            for h in range(NT):
                w_ = pw.tile([128, H], BF16, tag="pw", name=f"wv{h}")
                nc.sync.dma_start(out=w_, in_=wvt_d[h * 128:(h + 1) * 128, :])
                wv.append(w_)
            wo = []
            for h in range(NT):
                w_ = pw.tile([128, H], BF16, tag="pw", name=f"wo{h}")
                nc.sync.dma_start(out=w_, in_=wot_d[h * 128:(h + 1) * 128, :])
                wo.append(w_)
            ident = pms.tile([128, 128], BF16, tag="ms", name="ident")
            nc.sync.dma_start(out=ident, in_=id_d[:, :])

            if with_bq:
                bq_t = pms.tile([128, NT], F32, tag="ms", name="bq_t")
                nc.sync.dma_start(out=bq_t, in_=bq_d.rearrange("(j p) -> p j", p=128))
            if with_bk:
                bk_t = pms.tile([128, NT], F32, tag="ms", name="bk_t")
                nc.sync.dma_start(out=bk_t, in_=bk_d.rearrange("(j p) -> p j", p=128))
            if with_bv:
                # bv broadcast to all (Tq) partitions: [128, H] via DRAM roundtrip
                bv_r = pms.tile([1, H], F32, tag="ms", name="bv_r")
                nc.sync.dma_start(out=bv_r, in_=bv_d.rearrange("h -> 1 h"))
                bv_dr = dscr.tile([1, H], F32, tag="d", name="bv_dr")
                nc.sync.dma_start(out=bv_dr, in_=bv_r)
                bvb = pms.tile([128, H], F32, tag="ms", name="bvb")
                nc.sync.dma_start(out=bvb, in_=bv_dr.partition_broadcast(128))

            # ---- persistent result tiles ----
            qt = [pq.tile([128, T], BF16, tag="pq", name=f"qt{j}") for j in range(PAIRS)]
            kt = [pk.tile([128, T], BF16, tag="pk", name=f"kt{j}") for j in range(PAIRS)]
            vbuf = []
            for i in range(NT):
                vb = pv.tile([128, HEADS * 65], BF16, tag="pv", name=f"vb{i}")
                nc.vector.memset(
                    vb.rearrange("p (g c) -> p g c", c=65)[:, :, 64:65], 1.0)
                vbuf.append(vb)
            # ctx_sb[qb][t]: [128 Tq-sub, H] bf16 (normalized ctx, natural layout)
            ctx_sb = [[pcs.tile([128, H], BF16, tag="cs", name=f"cs{qb}{t}")
                       for t in range(4)] for qb in range(NB)]
            # ctxT[j]: [128 H-dims, T] bf16 (transposed ctx for out-proj)
            ctxT = [pct.tile([128, T], BF16, tag="ct", name=f"ctT{j}")
                    for j in range(PAIRS)]

            # ---- chain generators (each yields after one PE matmul) ----
            def qk_chain(kind, j, qb):
                ps = psF.tile([128, 512], F32, tag="f", name=f"{kind}ch{j}{qb}")
                wsrc = wq if kind == "q" else wk
                xsrc = xqt if kind == "q" else xkt
                for h in range(NT):
                    nc.tensor.matmul(
                        ps, wsrc[h][:, j * 128:(j + 1) * 128],
                        xsrc[h][:, qb * 512:(qb + 1) * 512],
                        start=(h == 0), stop=(h == NT - 1))
                    yield 213
                dst = (qt if kind == "q" else kt)[j][:, qb * 512:(qb + 1) * 512]
                if kind == "q" and with_bq:
                    nc.vector.tensor_scalar_add(dst, ps, bq_t[:, j:j + 1])
                elif kind == "k" and with_bk:
                    nc.vector.tensor_scalar_add(dst, ps, bk_t[:, j:j + 1])
                else:
                    nc.vector.tensor_copy(out=dst, in_=ps)

            def v_chain(blk, i):
                ps = psF.tile([128, 512], F32, tag="f", name=f"vch{blk}{i}")
                for h in range(NT):
                    nc.tensor.matmul(
                        ps, xkt[h][:, i * 128:(i + 1) * 128],
                        wv[h][:, blk * 512:(blk + 1) * 512],
                        start=(h == 0), stop=(h == NT - 1))
                    yield 213
                dst = vbuf[i][:, blk * 8 * 65:(blk + 1) * 8 * 65] \
                    .rearrange("p (g c) -> p g c", c=65)[:, :, 0:64]
                nc.vector.tensor_copy(
                    out=dst, in_=ps.rearrange("p (g d) -> p g d", d=64))

            def t_chain(qb, j, pool, ptag):
                # transpose ctx_sb[qb][t][:, j*128:(j+1)*128] -> ctxT[j][:, qb cols]
                ps = pool.tile([128, 512], F32, tag=ptag, name=f"tch{qb}{j}")
                pb = ps.bitcast(BF16)   # [128, 1024] bf16 view
                for t in range(4):
                    nc.tensor.matmul(
                        pb[:, t * 128:(t + 1) * 128],
                        ctx_sb[qb][t][:, j * 128:(j + 1) * 128],
                        ident, start=True, stop=True, is_transpose=True,
                        skip_group_check=True)
                    yield 53
                nc.vector.tensor_copy(
                    out=ctxT[j][:, qb * 512:(qb + 1) * 512], in_=pb[:, 0:512])

            def o_chain(tt, ob, pool, ptag):
                ps = pool.tile([128, 512], F32, tag=ptag, name=f"och{tt}{ob}")
                for j in range(NT):
                    nc.tensor.matmul(
                        ps, ctxT[j][:, tt * 128:(tt + 1) * 128],
                        wo[j][:, ob * 512:(ob + 1) * 512],
                        start=(j == 0), stop=(j == NT - 1))
                    yield 213
                o_ = pou.tile([128, 512], F32, tag="ou", name=f"ot{tt}{ob}")
                nc.vector.tensor_copy(out=o_, in_=ps)
                nc.sync.dma_start(
                    out=out_d[tt * 128:(tt + 1) * 128, ob * 512:(ob + 1) * 512],
                    in_=o_)

            fillers = deque()   # items: (tag, generator)
            done_tags = set()

            def pull(budget_ns):
                # advance fillers up to ~budget_ns of PE time (chains yield
                # their matmul cost), stopping at a chain boundary: the next
                # chain's first matmul would stall the in-order PE queue on
                # the finished chain's psum eviction.
                while budget_ns > 0 and fillers:
                    try:
                        budget_ns -= next(fillers[0][1])
                    except StopIteration:
                        done_tags.add(fillers[0][0])
                        fillers.popleft()
                        break

            def ensure(*tags):
                # force-drain fillers (in order) until all tags are emitted
                need = [t for t in tags if t not in done_tags]
                while need:
                    if not fillers:
                        raise AssertionError(f"filler tags missing: {need}")
                    tag, gen = fillers[0]
                    for _ in gen:
                        pass
                    done_tags.add(tag)
                    fillers.popleft()
                    need = [t for t in tags if t not in done_tags]

            def run_now(gen):
                for _ in gen:
                    pass

            # ---- pre-attention: 8-slot h-outer waves (all PSUM banks idle
            # here, so borrow psS+psC+psF for full DMA/compute pipelining) ----
            def wave(kind, wname):
                big = [psS.tile([128, 1024], F32, tag="s", name=f"w{wname}{c}")
                       for c in range(2)]
                small = [psC.tile([128, 512], F32, tag="c", name=f"w{wname}c{c}")
                         for c in range(3)]
                small.append(psF.tile([128, 512], F32, tag="f", name=f"w{wname}f"))
                slots = [big[0][:, 0:512], big[0][:, 512:1024],
                         big[1][:, 0:512], big[1][:, 512:1024]] + small
                if kind == "v":
                    # slot s = Tk-tile i=s, H-block 0 (pairs 0-3)
                    for h in range(NT):
                        for s in range(8):
                            nc.tensor.matmul(
                                slots[s], xkt[h][:, s * 128:(s + 1) * 128],
                                wv[h][:, 0:512],
                                start=(h == 0), stop=(h == NT - 1))
                    for s in range(8):
                        dst = vbuf[s][:, 0:8 * 65] \
                            .rearrange("p (g c) -> p g c", c=65)[:, :, 0:64]
                        nc.vector.tensor_copy(
                            out=dst, in_=slots[s].rearrange("p (g d) -> p g d", d=64))
                else:
                    wsrc = wq if kind == "q" else wk
                    xsrc = xqt if kind == "q" else xkt
                    chains = [(j, qb) for j in range(4) for qb in range(NB)]
                    for h in range(NT):
                        for s, (j, qb) in enumerate(chains):
                            nc.tensor.matmul(
                                slots[s], wsrc[h][:, j * 128:(j + 1) * 128],
                                xsrc[h][:, qb * 512:(qb + 1) * 512],
                                start=(h == 0), stop=(h == NT - 1))
                    for s, (j, qb) in enumerate(chains):
                        dst = (qt if kind == "q" else kt)[j][:, qb * 512:(qb + 1) * 512]
                        if kind == "q" and with_bq:
                            nc.vector.tensor_scalar_add(dst, slots[s], bq_t[:, j:j + 1])
                        elif kind == "k" and with_bk:
                            nc.vector.tensor_scalar_add(dst, slots[s], bk_t[:, j:j + 1])
                        else:
                            nc.vector.tensor_copy(out=dst, in_=slots[s])

            wave("q", "q")
            wave("k", "k")

            # remaining projections become fillers inside the attention loop;
            # V block 0 leads (the first unit's lagged ctx consumes it)
            for i in range(NT):
                fillers.append((f"v0.{i}", v_chain(0, i)))
            for qb in range(NB):
                fillers.append((f"q4{qb}", qk_chain("q", 4, qb)))
                fillers.append((f"k4{qb}", qk_chain("k", 4, qb)))
            for i in range(NT):
                fillers.append((f"v1.{i}", v_chain(1, i)))
            for j in (5, 6, 7):
                fillers.append((f"q{j}0", qk_chain("q", j, 0)))
                fillers.append((f"k{j}0", qk_chain("k", j, 0)))
                fillers.append((f"k{j}1", qk_chain("k", j, 1)))
            # qb1 halves of Q for pairs 5-7 are only needed by qb1 attention:
            # defer them (pushed after qb0's transpose/out-proj fillers below)
            deferred_q = [(f"q{j}1", qk_chain("q", j, 1)) for j in (5, 6, 7)]

            # ---- attention ----
            def emit_ctx(j, qb, i, ex, cH):
                for h in range(2):
                    g = 2 * j + h
                    for t in range(4):
                        nc.tensor.matmul(
                            cH[h][:, t * 65:t * 65 + 65],
                            ex[:, h * 512 + t * 128:h * 512 + (t + 1) * 128],
                            vbuf[i][:, g * 65:(g + 1) * 65],
                            start=(i == 0 and t == 0),
                            stop=(i == NT - 1 and t == 0),
                            skip_group_check=(t > 0))

            def normalize(j, qb, cH):
                for h in range(2):
                    g = 2 * j + h
                    rec = prc.tile([128, 4], F32, tag="rc", name=f"rc{qb}{j}{h}")
                    nc.vector.reciprocal(
                        out=rec.rearrange("p (t c) -> p t c", c=1),
                        in_=cH[h][:, 64:64 + 4 * 65]
                        .rearrange("p (t c) -> p t c", c=65)[:, :, 0:1])
                    for t in range(4):
                        dst = ctx_sb[qb][t][:, g * 64:(g + 1) * 64]
                        src = cH[h][:, t * 65:t * 65 + 64]
                        if with_bv:
                            nc.vector.scalar_tensor_tensor(
                                out=dst, in0=src, scalar=rec[:, t:t + 1],
                                in1=bvb[:, g * 64:(g + 1) * 64],
                                op0=mybir.AluOpType.mult,
                                op1=mybir.AluOpType.add)
                        else:
                            nc.vector.tensor_scalar_mul(dst, src, rec[:, t:t + 1])

            for qb in range(NB):
                for j in range(PAIRS):
                    if j >= 4:
                        ensure(f"q{j}{qb}", f"k{j}0", f"k{j}1",
                               *(f"v1.{i}" for i in range(NT)))
                    cH = [psC.tile([128, 512], F32, tag="c", name=f"c{qb}{j}{h}")
                          for h in range(2)]
                    exs = []
                    blk = j // 4
                    # the very first unit lags ctx by 4 so V block 0 (a filler
                    # chain here, not a pre-attention wave) has time to land
                    lag = 4 if (qb == 0 and j == 0) else 1
                    for i in range(NT):
                        # scores+exp first: a stalled filler matmul must never
                        # sit between the exp and the scores feeding it (ACT
                        # is the attention pacer); fillers absorb the slack
                        # after the exp is dispatched.
                        ss = psS.tile([128, 1024], F32, tag="s", name=f"ss{qb}{j}{i}")
                        for h in range(2):
                            r0 = 64 * h
                            nc.tensor.matmul(
                                ss[:, h * 512:(h + 1) * 512],
                                kt[j][r0:r0 + 64, i * 128:(i + 1) * 128],
                                qt[j][r0:r0 + 64, qb * 512:(qb + 1) * 512],
                                start=True, stop=True)
                        ex = pex.tile([128, 1024], BF16, tag="e", name=f"ex{qb}{j}{i}")
                        nc.scalar.activation(out=ex, in_=ss, func=AF.Exp)
                        exs.append(ex)
                        if i >= lag:
                            ensure(f"v{blk}.{i - lag}")
                            emit_ctx(j, qb, i - lag, exs[i - lag], cH)
                            pull(390)
                        else:
                            pull(610)
                    for i in range(NT - lag, NT):
                        ensure(f"v{blk}.{i}")
                        emit_ctx(j, qb, i, exs[i], cH)
                    normalize(j, qb, cH)
                    # this pair's ctx_sb columns are final: transpose chain
                    # becomes a filler right away
                    fillers.append((f"t{qb}{j}", t_chain(qb, j, psF, "f")))
                if qb == 0:
                    # deferred Q halves first: ensure() at (qb1, j>=5) must not
                    # force-drain the out-proj chains behind them
                    fillers.extend(deferred_q)
                # out-proj needs ctxT for all pairs (qb0's chains fill qb1's
                # attention on psF; qb1's run at the tail on psC's free banks)
                pool, ptag = (psF, "f") if qb == 0 else (psC, "c")
                for tt in range(qb * 4, (qb + 1) * 4):
                    for ob in range(NB):
                        fillers.append((f"o{tt}{ob}", o_chain(tt, ob, pool, ptag)))

            while fillers:
                pull(1 << 30)

    nc.finalize()
    return nc


def _prepare(inputs_q, inputs_kv, w_q, b_q, w_kv, b_kv, w_o, b_o,
             r_q, s_q, r_kv, s_kv, heads):
    inputs_q = np.asarray(inputs_q, np.float32)
    inputs_kv = np.asarray(inputs_kv, np.float32)
    w_q = np.asarray(w_q, np.float32)
    b_q = np.asarray(b_q, np.float32)
    w_kv = np.asarray(w_kv, np.float32)
    b_kv = np.asarray(b_kv, np.float32)
    w_o = np.asarray(w_o, np.float32)
    b_o = np.asarray(b_o, np.float32)
    r_q = np.asarray(r_q, np.float32)
    s_q = np.asarray(s_q, np.float32)
    r_kv = np.asarray(r_kv, np.float32)
    s_kv = np.asarray(s_kv, np.float32)
    heads = int(heads)
    assert heads == HEADS and inputs_q.shape == (T, B, H)

    scale = np.float32((H // heads) ** -0.5)

    # split w_kv / b_kv / s_kv into K and V parts (2H axis = heads x {k,v} x hd)
    w_kv_r = w_kv.reshape(HEADS, 2, HD, H)
    k_w = w_kv_r[:, 0].reshape(H, H)
    v_w = w_kv_r[:, 1].reshape(H, H)
    b_kv_r = b_kv.reshape(HEADS, 2, HD)
    bk = np.ascontiguousarray(b_kv_r[:, 0].reshape(H))
    bv = np.ascontiguousarray(b_kv_r[:, 1].reshape(H))
    s_kv_r = s_kv.reshape(B, HEADS, 2, HD)
    s_k = s_kv_r[:, :, 0].reshape(B, H)
    s_v = s_kv_r[:, :, 1].reshape(B, H)

    with_bq = bool(np.any(b_q))
    with_bk = bool(np.any(bk))
    with_bv = bool(np.any(bv))
    key = (with_bq, with_bk, with_bv)

    wot = np.ascontiguousarray(w_o.T).astype(NPBF)
    ident = np.eye(128, dtype=NPBF)
    in_maps = []
    for b in range(B):
        m = {
            "xqt": np.ascontiguousarray(inputs_q[:, b, :].T).astype(NPBF),
            "xkt": np.ascontiguousarray(inputs_kv[:, b, :].T).astype(NPBF),
            # W''[o,h] = s[o]*W[o,h]*r[h]; lhsT wants [h, o] = W''.T
            "wqt": np.ascontiguousarray(
                (w_q * (s_q[b] * scale)[:, None] * r_q[b][None, :]).T).astype(NPBF),
            "wkt": np.ascontiguousarray(
                (k_w * s_k[b][:, None] * r_kv[b][None, :]).T).astype(NPBF),
            "wvt": np.ascontiguousarray(
                (v_w * s_v[b][:, None] * r_kv[b][None, :]).T).astype(NPBF),
            "wot": wot,
            "ident": ident,
        }
        if with_bq:
            m["bq"] = b_q * scale
        if with_bk:
            m["bk"] = bk
        if with_bv:
            m["bv"] = bv
        in_maps.append(m)
    return key, in_maps


def kernel(inputs_q, inputs_kv, w_q, b_q, w_kv, b_kv, w_o, b_o,
           r_q, s_q, r_kv, s_kv, heads):
    b_o = np.asarray(b_o, np.float32)
    key, in_maps = _prepare(inputs_q, inputs_kv, w_q, b_q, w_kv, b_kv,
                            w_o, b_o, r_q, s_q, r_kv, s_kv, heads)
    if key not in _cache:
        _cache[key] = _build(*key)
    nc = _cache[key]

    global _last_in_maps
    _last_in_maps = in_maps
    res = run_bass_kernel_spmd(nc, in_maps, list(range(B)))
    out = np.empty((T, B, H), np.float32)
    for b in range(B):
        out[:, b, :] = res.results[b]["out"]
    out += b_o
    return out


# revision 42
# speedup vs baseline: 1.3385x; 1.0054x over previous
"""Trainium2 Bass kernel for BatchEnsemble encoder-decoder multihead attention.

Problem (hardcoded shapes): Tq=Tk=1024, B=8, H=1024, heads=16, hd=64.

Sharding: pure data parallelism - batch B=8 across the 8 NeuronCores, one
batch element per core. No collectives.

Per-core math (batch b), BatchEnsemble rank-1 factors and the 1/sqrt(hd)
scale folded into per-core bf16 weights on the host:
    Q^T = Wq''^T.T @ Xq^T          [H, Tq]  (head-pair dims on partitions)
    K^T = Wk''^T.T @ Xk^T          [H, Tk]
    V   = Xk^T.T @ Wv''^T          [Tk, H]  (natural layout, [V|1] stationary)
    per head g:
      S^T = K_g^T.T @ Q_g^T        [Tk, Tq]   bf16 matmuls
      E   = exp(S^T) -> bf16       (scores bounded, no max-subtraction)
      [ctx | den] = E_tile.T @ [V_g | 1]   [Tq-tile, 65]  <- transposed ctx:
            stationary = E tile (128 Tk x 128 Tq), moving = [V|1]: full PE
            utilization (the ones column gives the softmax denominator).
      ctx = ctx * (1/den)          (DVE per-partition scalar, no broadcast DMA)
    ctx^T via PE transpose (identity RHS, bf16)  -> [H, Tq]
    out  = ctx^T.T @ Wo^T          [Tq, H]

All matmul operands bf16 (1 cycle/row on the PE, fp32 PSUM accumulate);
measured end-to-end absmax error ~5e-3 of output scale.

Schedule: the attention inner loop is ACT(exp)-paced, so projection /
out-projection / transpose matmuls are emitted as fine-grained "fillers"
(a few matmuls per attention i-step) to keep the PE busy during exp waits.
ctx accumulators for 4 Tq-subtiles share one PSUM bank (regions at 65-col
offsets, lazy-zero semantics with skip_group_check on regions 1-3).
Out-projection for the first Tq half runs during the second half's
attention.
"""

from collections import deque

import numpy as np
import ml_dtypes

import concourse.bass as bass
import concourse.tile as tile
import concourse.mybir as mybir
from concourse import bacc
from concourse.bass_utils import run_bass_kernel_spmd

F32 = mybir.dt.float32
BF16 = mybir.dt.bfloat16
AF = mybir.ActivationFunctionType
NPBF = ml_dtypes.bfloat16

T = 1024        # Tq = Tk
H = 1024
B = 8
HEADS = 16
HD = 64
NT = T // 128   # 8 x 128-tiles
NB = T // 512   # 2 x 512-blocks (qb)
PAIRS = HEADS // 2

_cache = {}
_last_in_maps = None


def _build(with_bq, with_bk, with_bv):
    nc = bacc.Bacc("TRN2", target_bir_lowering=False, debug=False)

    xqt_d = nc.dram_tensor("xqt", [H, T], BF16, kind="ExternalInput")
    xkt_d = nc.dram_tensor("xkt", [H, T], BF16, kind="ExternalInput")
    wqt_d = nc.dram_tensor("wqt", [H, H], BF16, kind="ExternalInput")
    wkt_d = nc.dram_tensor("wkt", [H, H], BF16, kind="ExternalInput")
    wvt_d = nc.dram_tensor("wvt", [H, H], BF16, kind="ExternalInput")
    wot_d = nc.dram_tensor("wot", [H, H], BF16, kind="ExternalInput")
    id_d = nc.dram_tensor("ident", [128, 128], BF16, kind="ExternalInput")
    bq_d = nc.dram_tensor("bq", [H], F32, kind="ExternalInput") if with_bq else None
    bk_d = nc.dram_tensor("bk", [H], F32, kind="ExternalInput") if with_bk else None
    bv_d = nc.dram_tensor("bv", [H], F32, kind="ExternalInput") if with_bv else None
    out_d = nc.dram_tensor("out", [T, H], F32, kind="ExternalOutput")

    with tile.TileContext(nc) as tc:
        with tc.tile_pool(name="px", bufs=16) as px, \
             tc.tile_pool(name="pw", bufs=32) as pw, \
             tc.tile_pool(name="pq", bufs=8) as pq, \
             tc.tile_pool(name="pk", bufs=8) as pk, \
             tc.tile_pool(name="pv", bufs=8) as pv, \
             tc.tile_pool(name="pex", bufs=6) as pex, \
             tc.tile_pool(name="pcs", bufs=8) as pcs, \
             tc.tile_pool(name="pct", bufs=8) as pct, \
             tc.tile_pool(name="pou", bufs=3) as pou, \
             tc.tile_pool(name="prc", bufs=6) as prc, \
             tc.tile_pool(name="pms", bufs=4) as pms, \
             tc.tile_pool(name="dscr", bufs=2, space="DRAM") as dscr, \
             tc.tile_pool(name="psS", bufs=2, space="PSUM") as psS, \
             tc.tile_pool(name="psC", bufs=3, space="PSUM") as psC, \
             tc.tile_pool(name="psF", bufs=1, space="PSUM") as psF:

            # ---- input DMAs, consumption order ----
            def dma_in(tile_, src, h):
                # split the first h-tile into column halves so the first wave
                # matmul can start on half the bytes
                if h == 0:
                    nc.sync.dma_start(out=tile_[:, 0:512], in_=src[:, 0:512])
                    nc.sync.dma_start(out=tile_[:, 512:1024], in_=src[:, 512:1024])
                else:
                    nc.sync.dma_start(out=tile_, in_=src)

            # identity first: it feeds the PE warmup chain below
            ident = pms.tile([128, 128], BF16, tag="ms", name="ident")
            nc.sync.dma_start(out=ident, in_=id_d[:, :])

            xqt, wq = [], []
            for h in range(NT):
                t_ = px.tile([128, T], BF16, tag="px", name=f"xqt{h}")
                dma_in(t_, xqt_d[h * 128:(h + 1) * 128, :], h)
                xqt.append(t_)
                w_ = pw.tile([128, H], BF16, tag="pw", name=f"wq{h}")
                dma_in(w_, wqt_d[h * 128:(h + 1) * 128, :], h)
                wq.append(w_)
            xkt, wk = [], []
            for h in range(NT):
                t_ = px.tile([128, T], BF16, tag="px", name=f"xkt{h}")
                dma_in(t_, xkt_d[h * 128:(h + 1) * 128, :], h)
                xkt.append(t_)
                w_ = pw.tile([128, H], BF16, tag="pw", name=f"wk{h}")
                dma_in(w_, wkt_d[h * 128:(h + 1) * 128, :], h)
                wk.append(w_)
            wv = []
            for h in range(NT):
                w_ = pw.tile([128, H], BF16, tag="pw", name=f"wv{h}")
                nc.sync.dma_start(out=w_, in_=wvt_d[h * 128:(h + 1) * 128, :])
                wv.append(w_)
            wo = []
            for h in range(NT):
                w_ = pw.tile([128, H], BF16, tag="pw", name=f"wo{h}")
                nc.sync.dma_start(out=w_, in_=wot_d[h * 128:(h + 1) * 128, :])
                wo.append(w_)
            if with_bq:
                bq_t = pms.tile([128, NT], F32, tag="ms", name="bq_t")
                nc.sync.dma_start(out=bq_t, in_=bq_d.rearrange("(j p) -> p j", p=128))
            if with_bk:
                bk_t = pms.tile([128, NT], F32, tag="ms", name="bk_t")
                nc.sync.dma_start(out=bk_t, in_=bk_d.rearrange("(j p) -> p j", p=128))
            if with_bv:
                # bv broadcast to all (Tq) partitions: [128, H] via DRAM roundtrip
                bv_r = pms.tile([1, H], F32, tag="ms", name="bv_r")
                nc.sync.dma_start(out=bv_r, in_=bv_d.rearrange("h -> 1 h"))
                bv_dr = dscr.tile([1, H], F32, tag="d", name="bv_dr")
                nc.sync.dma_start(out=bv_dr, in_=bv_r)
                bvb = pms.tile([128, H], F32, tag="ms", name="bvb")
                nc.sync.dma_start(out=bvb, in_=bv_dr.partition_broadcast(128))

            # ---- persistent result tiles ----
            qt = [pq.tile([128, T], BF16, tag="pq", name=f"qt{j}") for j in range(PAIRS)]
            kt = [pk.tile([128, T], BF16, tag="pk", name=f"kt{j}") for j in range(PAIRS)]
            vbuf = []
            for i in range(NT):
                vb = pv.tile([128, HEADS * 65], BF16, tag="pv", name=f"vb{i}")
                nc.vector.memset(
                    vb.rearrange("p (g c) -> p g c", c=65)[:, :, 64:65], 1.0)
                vbuf.append(vb)
            # ctx_sb[qb][t]: [128 Tq-sub, H] bf16 (normalized ctx, natural layout)
            ctx_sb = [[pcs.tile([128, H], BF16, tag="cs", name=f"cs{qb}{t}")
                       for t in range(4)] for qb in range(NB)]
            # ctxT[j]: [128 H-dims, T] bf16 (transposed ctx for out-proj)
            ctxT = [pct.tile([128, T], BF16, tag="ct", name=f"ctT{j}")
                    for j in range(PAIRS)]

            # ---- chain generators (each yields after one PE matmul) ----
            def qk_chain(kind, j, qb):
                ps = psF.tile([128, 512], F32, tag="f", name=f"{kind}ch{j}{qb}")
                wsrc = wq if kind == "q" else wk
                xsrc = xqt if kind == "q" else xkt
                for h in range(NT):
                    nc.tensor.matmul(
                        ps, wsrc[h][:, j * 128:(j + 1) * 128],
                        xsrc[h][:, qb * 512:(qb + 1) * 512],
                        start=(h == 0), stop=(h == NT - 1))
                    yield 213
                dst = (qt if kind == "q" else kt)[j][:, qb * 512:(qb + 1) * 512]
                if kind == "q" and with_bq:
                    nc.vector.tensor_scalar_add(dst, ps, bq_t[:, j:j + 1])
                elif kind == "k" and with_bk:
                    nc.vector.tensor_scalar_add(dst, ps, bk_t[:, j:j + 1])
                else:
                    nc.vector.tensor_copy(out=dst, in_=ps)

            def v_chain(blk, i):
                ps = psF.tile([128, 512], F32, tag="f", name=f"vch{blk}{i}")
                for h in range(NT):
                    nc.tensor.matmul(
                        ps, xkt[h][:, i * 128:(i + 1) * 128],
                        wv[h][:, blk * 512:(blk + 1) * 512],
                        start=(h == 0), stop=(h == NT - 1))
                    yield 213
                dst = vbuf[i][:, blk * 8 * 65:(blk + 1) * 8 * 65] \
                    .rearrange("p (g c) -> p g c", c=65)[:, :, 0:64]
                nc.vector.tensor_copy(
                    out=dst, in_=ps.rearrange("p (g d) -> p g d", d=64))

            def t_chain(qb, j, pool, ptag):
                # transpose ctx_sb[qb][t][:, j*128:(j+1)*128] -> ctxT[j][:, qb cols]
                ps = pool.tile([128, 512], F32, tag=ptag, name=f"tch{qb}{j}")
                pb = ps.bitcast(BF16)   # [128, 1024] bf16 view
                for t in range(4):
                    nc.tensor.matmul(
                        pb[:, t * 128:(t + 1) * 128],
                        ctx_sb[qb][t][:, j * 128:(j + 1) * 128],
                        ident, start=True, stop=True, is_transpose=True,
                        skip_group_check=True)
                    yield 53
                nc.vector.tensor_copy(
                    out=ctxT[j][:, qb * 512:(qb + 1) * 512], in_=pb[:, 0:512])

            def o_chain(tt, ob, pool, ptag):
                ps = pool.tile([128, 512], F32, tag=ptag, name=f"och{tt}{ob}")
                for j in range(NT):
                    nc.tensor.matmul(
                        ps, ctxT[j][:, tt * 128:(tt + 1) * 128],
                        wo[j][:, ob * 512:(ob + 1) * 512],
                        start=(j == 0), stop=(j == NT - 1))
                    yield 213
                o_ = pou.tile([128, 512], F32, tag="ou", name=f"ot{tt}{ob}")
                nc.vector.tensor_copy(out=o_, in_=ps)
                nc.sync.dma_start(
                    out=out_d[tt * 128:(tt + 1) * 128, ob * 512:(ob + 1) * 512],
                    in_=o_)

            fillers = deque()   # items: (tag, generator)
            done_tags = set()

            def pull(budget_ns):
                # advance fillers up to ~budget_ns of PE time (chains yield
                # their matmul cost), stopping at a chain boundary: the next
                # chain's first matmul would stall the in-order PE queue on
                # the finished chain's psum eviction.
                while budget_ns > 0 and fillers:
                    try:
                        budget_ns -= next(fillers[0][1])
                    except StopIteration:
                        done_tags.add(fillers[0][0])
                        fillers.popleft()
                        break

            def ensure(*tags):
                # force-drain fillers (in order) until all tags are emitted
                need = [t for t in tags if t not in done_tags]
                while need:
                    if not fillers:
                        raise AssertionError(f"filler tags missing: {need}")
                    tag, gen = fillers[0]
                    for _ in gen:
                        pass
                    done_tags.add(tag)
                    fillers.popleft()
                    need = [t for t in tags if t not in done_tags]

            def run_now(gen):
                for _ in gen:
                    pass

            # ---- pre-attention: 8-slot h-outer waves (all PSUM banks idle
            # here, so borrow psS+psC+psF for full DMA/compute pipelining) ----
            def wave(kind, wname):
                big = [psS.tile([128, 1024], F32, tag="s", name=f"w{wname}{c}")
                       for c in range(2)]
                small = [psC.tile([128, 512], F32, tag="c", name=f"w{wname}c{c}")
                         for c in range(3)]
                small.append(psF.tile([128, 512], F32, tag="f", name=f"w{wname}f"))
                slots = [big[0][:, 0:512], big[0][:, 512:1024],
                         big[1][:, 0:512], big[1][:, 512:1024]] + small
                if kind == "v":
                    # slot s = Tk-tile i=s, H-block 0 (pairs 0-3)
                    for h in range(NT):
                        for s in range(8):
                            nc.tensor.matmul(
                                slots[s], xkt[h][:, s * 128:(s + 1) * 128],
                                wv[h][:, 0:512],
                                start=(h == 0), stop=(h == NT - 1))
                    for s in range(8):
                        dst = vbuf[s][:, 0:8 * 65] \
                            .rearrange("p (g c) -> p g c", c=65)[:, :, 0:64]
                        nc.vector.tensor_copy(
                            out=dst, in_=slots[s].rearrange("p (g d) -> p g d", d=64))
                else:
                    wsrc = wq if kind == "q" else wk
                    xsrc = xqt if kind == "q" else xkt
                    chains = [(j, qb) for j in range(4) for qb in range(NB)]
                    for h in range(NT):
                        for s, (j, qb) in enumerate(chains):
                            nc.tensor.matmul(
                                slots[s], wsrc[h][:, j * 128:(j + 1) * 128],
                                xsrc[h][:, qb * 512:(qb + 1) * 512],
                                start=(h == 0), stop=(h == NT - 1))
                    for s, (j, qb) in enumerate(chains):
                        dst = (qt if kind == "q" else kt)[j][:, qb * 512:(qb + 1) * 512]
                        if kind == "q" and with_bq:
                            nc.vector.tensor_scalar_add(dst, slots[s], bq_t[:, j:j + 1])
                        elif kind == "k" and with_bk:
                            nc.vector.tensor_scalar_add(dst, slots[s], bk_t[:, j:j + 1])
                        else:
                            nc.vector.tensor_copy(out=dst, in_=slots[s])

            # PE warmup: the cost model ramps the PE clock over the first
            # ~3us of continuous execution; burn the initial input-DMA wait
            # on identity matmuls so the projection waves run at full clock.
            wu = psF.tile([128, 512], F32, tag="f", name="warmup")
            for r in range(25):
                nc.tensor.matmul(wu[:, 0:128], ident, ident,
                                 start=True, stop=True, skip_group_check=True)

            wave("q", "q")
            wave("k", "k")

            # remaining projections become fillers inside the attention loop;
            # V block 0 leads (the first unit's lagged ctx consumes it)
            for i in range(NT):
                fillers.append((f"v0.{i}", v_chain(0, i)))
            for qb in range(NB):
                fillers.append((f"q4{qb}", qk_chain("q", 4, qb)))
                fillers.append((f"k4{qb}", qk_chain("k", 4, qb)))
            for i in range(NT):
                fillers.append((f"v1.{i}", v_chain(1, i)))
            for j in (5, 6, 7):
                fillers.append((f"q{j}0", qk_chain("q", j, 0)))
                fillers.append((f"k{j}0", qk_chain("k", j, 0)))
                fillers.append((f"k{j}1", qk_chain("k", j, 1)))
            # qb1 halves of Q for pairs 5-7 are only needed by qb1 attention:
            # defer them (pushed after qb0's transpose/out-proj fillers below)
            deferred_q = [(f"q{j}1", qk_chain("q", j, 1)) for j in (5, 6, 7)]

            # ---- attention ----
            def emit_ctx(j, qb, i, ex, cH):
                for h in range(2):
                    g = 2 * j + h
                    for t in range(4):
                        nc.tensor.matmul(
                            cH[h][:, t * 65:t * 65 + 65],
                            ex[:, h * 512 + t * 128:h * 512 + (t + 1) * 128],
                            vbuf[i][:, g * 65:(g + 1) * 65],
                            start=(i == 0 and t == 0),
                            stop=(i == NT - 1 and t == 0),
                            skip_group_check=(t > 0))

            def normalize(j, qb, cH):
                for h in range(2):
                    g = 2 * j + h
                    rec = prc.tile([128, 4], F32, tag="rc", name=f"rc{qb}{j}{h}")
                    nc.vector.reciprocal(
                        out=rec.rearrange("p (t c) -> p t c", c=1),
                        in_=cH[h][:, 64:64 + 4 * 65]
                        .rearrange("p (t c) -> p t c", c=65)[:, :, 0:1])
                    for t in range(4):
                        dst = ctx_sb[qb][t][:, g * 64:(g + 1) * 64]
                        src = cH[h][:, t * 65:t * 65 + 64]
                        if with_bv:
                            nc.vector.scalar_tensor_tensor(
                                out=dst, in0=src, scalar=rec[:, t:t + 1],
                                in1=bvb[:, g * 64:(g + 1) * 64],
                                op0=mybir.AluOpType.mult,
                                op1=mybir.AluOpType.add)
                        else:
                            nc.vector.tensor_scalar_mul(dst, src, rec[:, t:t + 1])

            for qb in range(NB):
                for j in range(PAIRS):
                    if j >= 4:
                        ensure(f"q{j}{qb}", f"k{j}0", f"k{j}1",
                               *(f"v1.{i}" for i in range(NT)))
                    cH = [psC.tile([128, 512], F32, tag="c", name=f"c{qb}{j}{h}")
                          for h in range(2)]
                    exs = []
                    blk = j // 4
                    # the very first unit lags ctx by 4 so V block 0 (a filler
                    # chain here, not a pre-attention wave) has time to land
                    lag = 4 if (qb == 0 and j == 0) else 1
                    for i in range(NT):
                        # scores+exp first: a stalled filler matmul must never
                        # sit between the exp and the scores feeding it (ACT
                        # is the attention pacer); fillers absorb the slack
                        # after the exp is dispatched.
                        ss = psS.tile([128, 1024], F32, tag="s", name=f"ss{qb}{j}{i}")
                        for h in range(2):
                            r0 = 64 * h
                            nc.tensor.matmul(
                                ss[:, h * 512:(h + 1) * 512],
                                kt[j][r0:r0 + 64, i * 128:(i + 1) * 128],
                                qt[j][r0:r0 + 64, qb * 512:(qb + 1) * 512],
                                start=True, stop=True)
                        ex = pex.tile([128, 1024], BF16, tag="e", name=f"ex{qb}{j}{i}")
                        nc.scalar.activation(out=ex, in_=ss, func=AF.Exp)
                        exs.append(ex)
                        if i >= lag:
                            ensure(f"v{blk}.{i - lag}")
                            emit_ctx(j, qb, i - lag, exs[i - lag], cH)
                            pull(390)
                        else:
                            pull(610)
                    for i in range(NT - lag, NT):
                        ensure(f"v{blk}.{i}")
                        emit_ctx(j, qb, i, exs[i], cH)
                    normalize(j, qb, cH)
                    # this pair's ctx_sb columns are final: transpose chain
                    # becomes a filler right away
                    fillers.append((f"t{qb}{j}", t_chain(qb, j, psF, "f")))
                if qb == 0:
                    # deferred Q halves first: ensure() at (qb1, j>=5) must not
                    # force-drain the out-proj chains behind them
                    fillers.extend(deferred_q)
                # out-proj needs ctxT for all pairs (qb0's chains fill qb1's
                # attention on psF; qb1's run at the tail on psC's free banks)
                pool, ptag = (psF, "f") if qb == 0 else (psC, "c")
                for tt in range(qb * 4, (qb + 1) * 4):
                    for ob in range(NB):
                        fillers.append((f"o{tt}{ob}", o_chain(tt, ob, pool, ptag)))

            while fillers:
                pull(1 << 30)

    nc.finalize()
    return nc


def _prepare(inputs_q, inputs_kv, w_q, b_q, w_kv, b_kv, w_o, b_o,
             r_q, s_q, r_kv, s_kv, heads):
    inputs_q = np.asarray(inputs_q, np.float32)
    inputs_kv = np.asarray(inputs_kv, np.float32)
    w_q = np.asarray(w_q, np.float32)
    b_q = np.asarray(b_q, np.float32)
    w_kv = np.asarray(w_kv, np.float32)
    b_kv = np.asarray(b_kv, np.float32)
    w_o = np.asarray(w_o, np.float32)
    b_o = np.asarray(b_o, np.float32)
    r_q = np.asarray(r_q, np.float32)
    s_q = np.asarray(s_q, np.float32)
    r_kv = np.asarray(r_kv, np.float32)
    s_kv = np.asarray(s_kv, np.float32)
    heads = int(heads)
    assert heads == HEADS and inputs_q.shape == (T, B, H)

    scale = np.float32((H // heads) ** -0.5)

    # split w_kv / b_kv / s_kv into K and V parts (2H axis = heads x {k,v} x hd)
    w_kv_r = w_kv.reshape(HEADS, 2, HD, H)
    k_w = w_kv_r[:, 0].reshape(H, H)
    v_w = w_kv_r[:, 1].reshape(H, H)
    b_kv_r = b_kv.reshape(HEADS, 2, HD)
    bk = np.ascontiguousarray(b_kv_r[:, 0].reshape(H))
    bv = np.ascontiguousarray(b_kv_r[:, 1].reshape(H))
    s_kv_r = s_kv.reshape(B, HEADS, 2, HD)
    s_k = s_kv_r[:, :, 0].reshape(B, H)
    s_v = s_kv_r[:, :, 1].reshape(B, H)

    with_bq = bool(np.any(b_q))
    with_bk = bool(np.any(bk))
    with_bv = bool(np.any(bv))
    key = (with_bq, with_bk, with_bv)

    wot = np.ascontiguousarray(w_o.T).astype(NPBF)
    ident = np.eye(128, dtype=NPBF)
    in_maps = []
    for b in range(B):
        m = {
            "xqt": np.ascontiguousarray(inputs_q[:, b, :].T).astype(NPBF),
            "xkt": np.ascontiguousarray(inputs_kv[:, b, :].T).astype(NPBF),
            # W''[o,h] = s[o]*W[o,h]*r[h]; lhsT wants [h, o] = W''.T
            "wqt": np.ascontiguousarray(
                (w_q * (s_q[b] * scale)[:, None] * r_q[b][None, :]).T).astype(NPBF),
            "wkt": np.ascontiguousarray(
                (k_w * s_k[b][:, None] * r_kv[b][None, :]).T).astype(NPBF),
            "wvt": np.ascontiguousarray(
                (v_w * s_v[b][:, None] * r_kv[b][None, :]).T).astype(NPBF),
            "wot": wot,
            "ident": ident,
        }
        if with_bq:
            m["bq"] = b_q * scale
        if with_bk:
            m["bk"] = bk
        if with_bv:
            m["bv"] = bv
        in_maps.append(m)
    return key, in_maps


def kernel(inputs_q, inputs_kv, w_q, b_q, w_kv, b_kv, w_o, b_o,
           r_q, s_q, r_kv, s_kv, heads):
    b_o = np.asarray(b_o, np.float32)
    key, in_maps = _prepare(inputs_q, inputs_kv, w_q, b_q, w_kv, b_kv,
                            w_o, b_o, r_q, s_q, r_kv, s_kv, heads)
    if key not in _cache:
        _cache[key] = _build(*key)
    nc = _cache[key]

    global _last_in_maps
    _last_in_maps = in_maps
    res = run_bass_kernel_spmd(nc, in_maps, list(range(B)))
    out = np.empty((T, B, H), np.float32)
    for b in range(B):
        out[:, b, :] = res.results[b]["out"]
    out += b_o
    return out


# revision 45
# speedup vs baseline: 1.3472x; 1.0065x over previous
"""Trainium2 Bass kernel for BatchEnsemble encoder-decoder multihead attention.

Problem (hardcoded shapes): Tq=Tk=1024, B=8, H=1024, heads=16, hd=64.

Sharding: pure data parallelism - batch B=8 across the 8 NeuronCores, one
batch element per core. No collectives.

Per-core math (batch b), BatchEnsemble rank-1 factors and the 1/sqrt(hd)
scale folded into per-core bf16 weights on the host:
    Q^T = Wq''^T.T @ Xq^T          [H, Tq]  (head-pair dims on partitions)
    K^T = Wk''^T.T @ Xk^T          [H, Tk]
    V   = Xk^T.T @ Wv''^T          [Tk, H]  (natural layout, [V|1] stationary)
    per head g:
      S^T = K_g^T.T @ Q_g^T        [Tk, Tq]   bf16 matmuls
      E   = exp(S^T) -> bf16       (scores bounded, no max-subtraction)
      [ctx | den] = E_tile.T @ [V_g | 1]   [Tq-tile, 65]  <- transposed ctx:
            stationary = E tile (128 Tk x 128 Tq), moving = [V|1]: full PE
            utilization (the ones column gives the softmax denominator).
      ctx = ctx * (1/den)          (DVE per-partition scalar, no broadcast DMA)
    ctx^T via PE transpose (identity RHS, bf16)  -> [H, Tq]
    out  = ctx^T.T @ Wo^T          [Tq, H]

All matmul operands bf16 (1 cycle/row on the PE, fp32 PSUM accumulate);
measured end-to-end absmax error ~5e-3 of output scale.

Schedule: the attention inner loop is ACT(exp)-paced, so projection /
out-projection / transpose matmuls are emitted as fine-grained "fillers"
(a few matmuls per attention i-step) to keep the PE busy during exp waits.
ctx accumulators for 4 Tq-subtiles share one PSUM bank (regions at 65-col
offsets, lazy-zero semantics with skip_group_check on regions 1-3).
Out-projection for the first Tq half runs during the second half's
attention.
"""

from collections import deque

import numpy as np
import ml_dtypes

import concourse.bass as bass
import concourse.tile as tile
import concourse.mybir as mybir
from concourse import bacc
from concourse.bass_utils import run_bass_kernel_spmd

F32 = mybir.dt.float32
BF16 = mybir.dt.bfloat16
AF = mybir.ActivationFunctionType
NPBF = ml_dtypes.bfloat16

T = 1024        # Tq = Tk
H = 1024
B = 8
HEADS = 16
HD = 64
NT = T // 128   # 8 x 128-tiles
NB = T // 512   # 2 x 512-blocks (qb)
PAIRS = HEADS // 2

_cache = {}
_last_in_maps = None


def _build(with_bq, with_bk, with_bv):
    nc = bacc.Bacc("TRN2", target_bir_lowering=False, debug=False)

    xqt_d = nc.dram_tensor("xqt", [H, T], BF16, kind="ExternalInput")
    xkt_d = nc.dram_tensor("xkt", [H, T], BF16, kind="ExternalInput")
    wqt_d = nc.dram_tensor("wqt", [H, H], BF16, kind="ExternalInput")
    wkt_d = nc.dram_tensor("wkt", [H, H], BF16, kind="ExternalInput")
    wvt_d = nc.dram_tensor("wvt", [H, H], BF16, kind="ExternalInput")
    wot_d = nc.dram_tensor("wot", [H, H], BF16, kind="ExternalInput")
    id_d = nc.dram_tensor("ident", [128, 128], BF16, kind="ExternalInput")
    bq_d = nc.dram_tensor("bq", [H], F32, kind="ExternalInput") if with_bq else None
    bk_d = nc.dram_tensor("bk", [H], F32, kind="ExternalInput") if with_bk else None
    bv_d = nc.dram_tensor("bv", [H], F32, kind="ExternalInput") if with_bv else None
    out_d = nc.dram_tensor("out", [T, H], F32, kind="ExternalOutput")

    with tile.TileContext(nc) as tc:
        with tc.tile_pool(name="px", bufs=16) as px, \
             tc.tile_pool(name="pw", bufs=32) as pw, \
             tc.tile_pool(name="pq", bufs=8) as pq, \
             tc.tile_pool(name="pk", bufs=8) as pk, \
             tc.tile_pool(name="pv", bufs=8) as pv, \
             tc.tile_pool(name="pex", bufs=6) as pex, \
             tc.tile_pool(name="pcs", bufs=8) as pcs, \
             tc.tile_pool(name="pct", bufs=8) as pct, \
             tc.tile_pool(name="pou", bufs=3) as pou, \
             tc.tile_pool(name="prc", bufs=6) as prc, \
             tc.tile_pool(name="pms", bufs=4) as pms, \
             tc.tile_pool(name="dscr", bufs=2, space="DRAM") as dscr, \
             tc.tile_pool(name="psS", bufs=2, space="PSUM") as psS, \
             tc.tile_pool(name="psC", bufs=3, space="PSUM") as psC, \
             tc.tile_pool(name="psF", bufs=1, space="PSUM") as psF:

            # ---- input DMAs, consumption order ----
            def dma_in(tile_, src, h):
                # split the first h-tile into column halves so the first wave
                # matmul can start on half the bytes
                if h == 0:
                    nc.sync.dma_start(out=tile_[:, 0:512], in_=src[:, 0:512])
                    nc.sync.dma_start(out=tile_[:, 512:1024], in_=src[:, 512:1024])
                else:
                    nc.sync.dma_start(out=tile_, in_=src)

            # identity first: it feeds the PE warmup chain below
            ident = pms.tile([128, 128], BF16, tag="ms", name="ident")
            nc.sync.dma_start(out=ident, in_=id_d[:, :])

            xqt, wq = [], []
            for h in range(NT):
                t_ = px.tile([128, T], BF16, tag="px", name=f"xqt{h}")
                dma_in(t_, xqt_d[h * 128:(h + 1) * 128, :], h)
                xqt.append(t_)
                w_ = pw.tile([128, H], BF16, tag="pw", name=f"wq{h}")
                dma_in(w_, wqt_d[h * 128:(h + 1) * 128, :], h)
                wq.append(w_)
            xkt, wk = [], []
            for h in range(NT):
                t_ = px.tile([128, T], BF16, tag="px", name=f"xkt{h}")
                dma_in(t_, xkt_d[h * 128:(h + 1) * 128, :], h)
                xkt.append(t_)
                w_ = pw.tile([128, H], BF16, tag="pw", name=f"wk{h}")
                dma_in(w_, wkt_d[h * 128:(h + 1) * 128, :], h)
                wk.append(w_)
            wv = []
            for h in range(NT):
                w_ = pw.tile([128, H], BF16, tag="pw", name=f"wv{h}")
                nc.sync.dma_start(out=w_, in_=wvt_d[h * 128:(h + 1) * 128, :])
                wv.append(w_)
            wo = []
            for h in range(NT):
                w_ = pw.tile([128, H], BF16, tag="pw", name=f"wo{h}")
                nc.sync.dma_start(out=w_, in_=wot_d[h * 128:(h + 1) * 128, :])
                wo.append(w_)
            if with_bq:
                bq_t = pms.tile([128, NT], F32, tag="ms", name="bq_t")
                nc.sync.dma_start(out=bq_t, in_=bq_d.rearrange("(j p) -> p j", p=128))
            if with_bk:
                bk_t = pms.tile([128, NT], F32, tag="ms", name="bk_t")
                nc.sync.dma_start(out=bk_t, in_=bk_d.rearrange("(j p) -> p j", p=128))
            if with_bv:
                # bv broadcast to all (Tq) partitions: [128, H] via DRAM roundtrip
                bv_r = pms.tile([1, H], F32, tag="ms", name="bv_r")
                nc.sync.dma_start(out=bv_r, in_=bv_d.rearrange("h -> 1 h"))
                bv_dr = dscr.tile([1, H], F32, tag="d", name="bv_dr")
                nc.sync.dma_start(out=bv_dr, in_=bv_r)
                bvb = pms.tile([128, H], F32, tag="ms", name="bvb")
                nc.sync.dma_start(out=bvb, in_=bv_dr.partition_broadcast(128))

            # ---- persistent result tiles ----
            qt = [pq.tile([128, T], BF16, tag="pq", name=f"qt{j}") for j in range(PAIRS)]
            kt = [pk.tile([128, T], BF16, tag="pk", name=f"kt{j}") for j in range(PAIRS)]
            vbuf = []
            for i in range(NT):
                vb = pv.tile([128, HEADS * 65], BF16, tag="pv", name=f"vb{i}")
                nc.vector.memset(
                    vb.rearrange("p (g c) -> p g c", c=65)[:, :, 64:65], 1.0)
                vbuf.append(vb)
            # ctx_sb[qb][t]: [128 Tq-sub, H] bf16 (normalized ctx, natural layout)
            ctx_sb = [[pcs.tile([128, H], BF16, tag="cs", name=f"cs{qb}{t}")
                       for t in range(4)] for qb in range(NB)]
            # ctxT[j]: [128 H-dims, T] bf16 (transposed ctx for out-proj)
            ctxT = [pct.tile([128, T], BF16, tag="ct", name=f"ctT{j}")
                    for j in range(PAIRS)]

            # ---- chain generators (each yields after one PE matmul) ----
            def qk_chain(kind, j, qb):
                ps = psF.tile([128, 512], F32, tag="f", name=f"{kind}ch{j}{qb}")
                wsrc = wq if kind == "q" else wk
                xsrc = xqt if kind == "q" else xkt
                for h in range(NT):
                    nc.tensor.matmul(
                        ps, wsrc[h][:, j * 128:(j + 1) * 128],
                        xsrc[h][:, qb * 512:(qb + 1) * 512],
                        start=(h == 0), stop=(h == NT - 1))
                    yield 213
                dst = (qt if kind == "q" else kt)[j][:, qb * 512:(qb + 1) * 512]
                if kind == "q" and with_bq:
                    nc.vector.tensor_scalar_add(dst, ps, bq_t[:, j:j + 1])
                elif kind == "k" and with_bk:
                    nc.vector.tensor_scalar_add(dst, ps, bk_t[:, j:j + 1])
                else:
                    nc.vector.tensor_copy(out=dst, in_=ps)

            def v_chain(blk, i):
                ps = psF.tile([128, 512], F32, tag="f", name=f"vch{blk}{i}")
                for h in range(NT):
                    nc.tensor.matmul(
                        ps, xkt[h][:, i * 128:(i + 1) * 128],
                        wv[h][:, blk * 512:(blk + 1) * 512],
                        start=(h == 0), stop=(h == NT - 1))
                    yield 213
                dst = vbuf[i][:, blk * 8 * 65:(blk + 1) * 8 * 65] \
                    .rearrange("p (g c) -> p g c", c=65)[:, :, 0:64]
                nc.vector.tensor_copy(
                    out=dst, in_=ps.rearrange("p (g d) -> p g d", d=64))

            def t_chain(qb, j, pool, ptag):
                # transpose ctx_sb[qb][t][:, j*128:(j+1)*128] -> ctxT[j][:, qb cols]
                ps = pool.tile([128, 512], F32, tag=ptag, name=f"tch{qb}{j}")
                pb = ps.bitcast(BF16)   # [128, 1024] bf16 view
                for t in range(4):
                    nc.tensor.matmul(
                        pb[:, t * 128:(t + 1) * 128],
                        ctx_sb[qb][t][:, j * 128:(j + 1) * 128],
                        ident, start=True, stop=True, is_transpose=True,
                        skip_group_check=True)
                    yield 53
                nc.vector.tensor_copy(
                    out=ctxT[j][:, qb * 512:(qb + 1) * 512], in_=pb[:, 0:512])

            def o_chain(tt, ob, pool, ptag):
                ps = pool.tile([128, 512], F32, tag=ptag, name=f"och{tt}{ob}")
                for j in range(NT):
                    nc.tensor.matmul(
                        ps, ctxT[j][:, tt * 128:(tt + 1) * 128],
                        wo[j][:, ob * 512:(ob + 1) * 512],
                        start=(j == 0), stop=(j == NT - 1))
                    yield 213
                o_ = pou.tile([128, 512], F32, tag="ou", name=f"ot{tt}{ob}")
                nc.vector.tensor_copy(out=o_, in_=ps)
                nc.sync.dma_start(
                    out=out_d[tt * 128:(tt + 1) * 128, ob * 512:(ob + 1) * 512],
                    in_=o_)

            fillers = deque()   # items: (tag, generator)
            done_tags = set()

            def pull(budget_ns):
                # advance fillers up to ~budget_ns of PE time (chains yield
                # their matmul cost), stopping at a chain boundary: the next
                # chain's first matmul would stall the in-order PE queue on
                # the finished chain's psum eviction.
                while budget_ns > 0 and fillers:
                    try:
                        budget_ns -= next(fillers[0][1])
                    except StopIteration:
                        done_tags.add(fillers[0][0])
                        fillers.popleft()
                        break

            def ensure(*tags):
                # force-drain fillers (in order) until all tags are emitted
                need = [t for t in tags if t not in done_tags]
                while need:
                    if not fillers:
                        raise AssertionError(f"filler tags missing: {need}")
                    tag, gen = fillers[0]
                    for _ in gen:
                        pass
                    done_tags.add(tag)
                    fillers.popleft()
                    need = [t for t in tags if t not in done_tags]

            def run_now(gen):
                for _ in gen:
                    pass

            # ---- pre-attention: 8-slot h-outer waves (all PSUM banks idle
            # here, so borrow psS+psC+psF for full DMA/compute pipelining) ----
            def wave(kind, wname):
                big = [psS.tile([128, 1024], F32, tag="s", name=f"w{wname}{c}")
                       for c in range(2)]
                small = [psC.tile([128, 512], F32, tag="c", name=f"w{wname}c{c}")
                         for c in range(3)]
                small.append(psF.tile([128, 512], F32, tag="f", name=f"w{wname}f"))
                slots = [big[0][:, 0:512], big[0][:, 512:1024],
                         big[1][:, 0:512], big[1][:, 512:1024]] + small
                if kind == "v":
                    # slot s = Tk-tile i=s, H-block 0 (pairs 0-3)
                    for h in range(NT):
                        for s in range(8):
                            nc.tensor.matmul(
                                slots[s], xkt[h][:, s * 128:(s + 1) * 128],
                                wv[h][:, 0:512],
                                start=(h == 0), stop=(h == NT - 1))
                    for s in range(8):
                        dst = vbuf[s][:, 0:8 * 65] \
                            .rearrange("p (g c) -> p g c", c=65)[:, :, 0:64]
                        nc.vector.tensor_copy(
                            out=dst, in_=slots[s].rearrange("p (g d) -> p g d", d=64))
                else:
                    wsrc = wq if kind == "q" else wk
                    xsrc = xqt if kind == "q" else xkt
                    chains = [(j, qb) for j in range(4) for qb in range(NB)]
                    for h in range(NT):
                        for s, (j, qb) in enumerate(chains):
                            nc.tensor.matmul(
                                slots[s], wsrc[h][:, j * 128:(j + 1) * 128],
                                xsrc[h][:, qb * 512:(qb + 1) * 512],
                                start=(h == 0), stop=(h == NT - 1))
                    for s, (j, qb) in enumerate(chains):
                        dst = (qt if kind == "q" else kt)[j][:, qb * 512:(qb + 1) * 512]
                        if kind == "q" and with_bq:
                            nc.vector.tensor_scalar_add(dst, slots[s], bq_t[:, j:j + 1])
                        elif kind == "k" and with_bk:
                            nc.vector.tensor_scalar_add(dst, slots[s], bk_t[:, j:j + 1])
                        else:
                            nc.vector.tensor_copy(out=dst, in_=slots[s])

            # PE warmup: the cost model ramps the PE clock over the first
            # ~3us of continuous execution; burn the initial input-DMA wait
            # on identity matmuls so the projection waves run at full clock.
            wu = psF.tile([128, 512], F32, tag="f", name="warmup")
            for r in range(25):
                nc.tensor.matmul(wu[:, 0:128], ident, ident,
                                 start=True, stop=True, skip_group_check=True)

            wave("q", "q")
            wave("k", "k")

            # remaining projections become fillers inside the attention loop;
            # V block 0 leads (the first unit's lagged ctx consumes it)
            for i in range(NT):
                fillers.append((f"v0.{i}", v_chain(0, i)))
            for qb in range(NB):
                fillers.append((f"q4{qb}", qk_chain("q", 4, qb)))
                fillers.append((f"k4{qb}", qk_chain("k", 4, qb)))
            for i in range(NT):
                fillers.append((f"v1.{i}", v_chain(1, i)))
            for j in (5, 6, 7):
                fillers.append((f"q{j}0", qk_chain("q", j, 0)))
                fillers.append((f"k{j}0", qk_chain("k", j, 0)))
                fillers.append((f"k{j}1", qk_chain("k", j, 1)))
            # qb1 halves of Q for pairs 5-7 are only needed by qb1 attention:
            # defer them (pushed after qb0's transpose/out-proj fillers below)
            deferred_q = [(f"q{j}1", qk_chain("q", j, 1)) for j in (5, 6, 7)]

            # ---- attention ----
            def emit_ctx(j, qb, i, ex, cH):
                for h in range(2):
                    g = 2 * j + h
                    for t in range(4):
                        nc.tensor.matmul(
                            cH[h][:, t * 65:t * 65 + 65],
                            ex[:, h * 512 + t * 128:h * 512 + (t + 1) * 128],
                            vbuf[i][:, g * 65:(g + 1) * 65],
                            start=(i == 0 and t == 0),
                            stop=(i == NT - 1 and t == 0),
                            skip_group_check=(t > 0))

            def normalize(j, qb, cH):
                for h in range(2):
                    g = 2 * j + h
                    rec = prc.tile([128, 4], F32, tag="rc", name=f"rc{qb}{j}{h}")
                    nc.vector.reciprocal(
                        out=rec.rearrange("p (t c) -> p t c", c=1),
                        in_=cH[h][:, 64:64 + 4 * 65]
                        .rearrange("p (t c) -> p t c", c=65)[:, :, 0:1])
                    for t in range(4):
                        dst = ctx_sb[qb][t][:, g * 64:(g + 1) * 64]
                        src = cH[h][:, t * 65:t * 65 + 64]
                        if with_bv:
                            nc.vector.scalar_tensor_tensor(
                                out=dst, in0=src, scalar=rec[:, t:t + 1],
                                in1=bvb[:, g * 64:(g + 1) * 64],
                                op0=mybir.AluOpType.mult,
                                op1=mybir.AluOpType.add)
                        else:
                            nc.vector.tensor_scalar_mul(dst, src, rec[:, t:t + 1])

            # attention units pipeline ACROSS unit boundaries: each unit's
            # trailing ctx group(s) + normalize are deferred into the next
            # unit, emitted only after its first scores/exp are dispatched —
            # the trailing ctx waits on the last exp and must not block the
            # next unit's (already runnable) scores in the in-order PE queue.
            pending = [None]

            def flush_pending():
                if pending[0] is not None:
                    pending[0]()
                    pending[0] = None

            for qb in range(NB):
                for j in range(PAIRS):
                    if j >= 4:
                        ensure(f"q{j}{qb}", f"k{j}0", f"k{j}1",
                               *(f"v1.{i}" for i in range(NT)))
                    cH = None   # allocated lazily after the previous unit's
                    exs = []    # deferred normalize is emitted (psC bufs=3)
                    blk = j // 4
                    # the very first unit lags ctx by 4 so V block 0 (a filler
                    # chain here, not a pre-attention wave) has time to land
                    lag = 4 if (qb == 0 and j == 0) else 1
                    for i in range(NT):
                        # scores+exp first: a stalled filler matmul must never
                        # sit between the exp and the scores feeding it (ACT
                        # is the attention pacer); fillers absorb the slack
                        # after the exp is dispatched.
                        ss = psS.tile([128, 1024], F32, tag="s", name=f"ss{qb}{j}{i}")
                        for h in range(2):
                            r0 = 64 * h
                            nc.tensor.matmul(
                                ss[:, h * 512:(h + 1) * 512],
                                kt[j][r0:r0 + 64, i * 128:(i + 1) * 128],
                                qt[j][r0:r0 + 64, qb * 512:(qb + 1) * 512],
                                start=True, stop=True)
                        ex = pex.tile([128, 1024], BF16, tag="e", name=f"ex{qb}{j}{i}")
                        nc.scalar.activation(out=ex, in_=ss, func=AF.Exp)
                        exs.append(ex)
                        if i == 0:
                            flush_pending()
                        if i >= lag:
                            if cH is None:
                                cH = [psC.tile([128, 512], F32, tag="c",
                                               name=f"c{qb}{j}{h}")
                                      for h in range(2)]
                            ensure(f"v{blk}.{i - lag}")
                            emit_ctx(j, qb, i - lag, exs[i - lag], cH)
                            pull(390)
                        else:
                            pull(610)

                    def tail_unit(j=j, qb=qb, cH=cH, exs=exs, blk=blk, lag=lag):
                        for i in range(NT - lag, NT):
                            ensure(f"v{blk}.{i}")
                            emit_ctx(j, qb, i, exs[i], cH)
                        normalize(j, qb, cH)
                        # this pair's ctx_sb columns are final: its transpose
                        # chain becomes a filler right away
                        fillers.append((f"t{qb}{j}", t_chain(qb, j, psF, "f")))
                    pending[0] = tail_unit
                flush_pending()
                if qb == 0:
                    # deferred Q halves first: ensure() at (qb1, j>=5) must not
                    # force-drain the out-proj chains behind them
                    fillers.extend(deferred_q)
                # out-proj needs ctxT for all pairs (qb0's chains fill qb1's
                # attention on psF; qb1's run at the tail on psC's free banks)
                pool, ptag = (psF, "f") if qb == 0 else (psC, "c")
                for tt in range(qb * 4, (qb + 1) * 4):
                    for ob in range(NB):
                        fillers.append((f"o{tt}{ob}", o_chain(tt, ob, pool, ptag)))

            while fillers:
                pull(1 << 30)

    nc.finalize()
    return nc


def _prepare(inputs_q, inputs_kv, w_q, b_q, w_kv, b_kv, w_o, b_o,
             r_q, s_q, r_kv, s_kv, heads):
    inputs_q = np.asarray(inputs_q, np.float32)
    inputs_kv = np.asarray(inputs_kv, np.float32)
    w_q = np.asarray(w_q, np.float32)
    b_q = np.asarray(b_q, np.float32)
    w_kv = np.asarray(w_kv, np.float32)
    b_kv = np.asarray(b_kv, np.float32)
    w_o = np.asarray(w_o, np.float32)
    b_o = np.asarray(b_o, np.float32)
    r_q = np.asarray(r_q, np.float32)
    s_q = np.asarray(s_q, np.float32)
    r_kv = np.asarray(r_kv, np.float32)
    s_kv = np.asarray(s_kv, np.float32)
    heads = int(heads)
    assert heads == HEADS and inputs_q.shape == (T, B, H)

    scale = np.float32((H // heads) ** -0.5)

    # split w_kv / b_kv / s_kv into K and V parts (2H axis = heads x {k,v} x hd)
    w_kv_r = w_kv.reshape(HEADS, 2, HD, H)
    k_w = w_kv_r[:, 0].reshape(H, H)
    v_w = w_kv_r[:, 1].reshape(H, H)
    b_kv_r = b_kv.reshape(HEADS, 2, HD)
    bk = np.ascontiguousarray(b_kv_r[:, 0].reshape(H))
    bv = np.ascontiguousarray(b_kv_r[:, 1].reshape(H))
    s_kv_r = s_kv.reshape(B, HEADS, 2, HD)
    s_k = s_kv_r[:, :, 0].reshape(B, H)
    s_v = s_kv_r[:, :, 1].reshape(B, H)

    with_bq = bool(np.any(b_q))
    with_bk = bool(np.any(bk))
    with_bv = bool(np.any(bv))
    key = (with_bq, with_bk, with_bv)

    wot = np.ascontiguousarray(w_o.T).astype(NPBF)
    ident = np.eye(128, dtype=NPBF)
    in_maps = []
    for b in range(B):
        m = {
            "xqt": np.ascontiguousarray(inputs_q[:, b, :].T).astype(NPBF),
            "xkt": np.ascontiguousarray(inputs_kv[:, b, :].T).astype(NPBF),
            # W''[o,h] = s[o]*W[o,h]*r[h]; lhsT wants [h, o] = W''.T
            "wqt": np.ascontiguousarray(
                (w_q * (s_q[b] * scale)[:, None] * r_q[b][None, :]).T).astype(NPBF),
            "wkt": np.ascontiguousarray(
                (k_w * s_k[b][:, None] * r_kv[b][None, :]).T).astype(NPBF),
            "wvt": np.ascontiguousarray(
                (v_w * s_v[b][:, None] * r_kv[b][None, :]).T).astype(NPBF),
            "wot": wot,
            "ident": ident,
        }
        if with_bq:
            m["bq"] = b_q * scale
        if with_bk:
            m["bk"] = bk
        if with_bv:
            m["bv"] = bv
        in_maps.append(m)
    return key, in_maps


def kernel(inputs_q, inputs_kv, w_q, b_q, w_kv, b_kv, w_o, b_o,
           r_q, s_q, r_kv, s_kv, heads):
    b_o = np.asarray(b_o, np.float32)
    key, in_maps = _prepare(inputs_q, inputs_kv, w_q, b_q, w_kv, b_kv,
                            w_o, b_o, r_q, s_q, r_kv, s_kv, heads)
    if key not in _cache:
        _cache[key] = _build(*key)
    nc = _cache[key]

    global _last_in_maps
    _last_in_maps = in_maps
    res = run_bass_kernel_spmd(nc, in_maps, list(range(B)))
    out = np.empty((T, B, H), np.float32)
    for b in range(B):
        out[:, b, :] = res.results[b]["out"]
    out += b_o
    return out
